# revision 16
# baseline (speedup 1.0000x reference)
"""Trainium2 Bass kernel for nn_CommNetActor.

Network (per sample, 4 agents, all weights shared across agents):
    H0 = sigmoid(O @ enc_w + enc_b)            [B,4,128]
    H1..H3 = relu chain of 128x128 fc layers
    C  = (sum_j H3[:,j] - H3) / 4              (CommNet neighbour mean)
    H4 = [H3 | C] @ cl4_w + cl4_b
    logits = H4.reshape(B,512) @ dec_w + dec_b
    out = softmax(logits)                      [B,16]

Key algebraic fold used here: since C is linear in the H3 agent slices,
the whole tail (neighbour mean + cl4 + dec) collapses into per-agent
readout matrices applied directly to H3:
    logits[b] = sum_a H3[b,a] @ Wz_a + bias'
    Wz_a  = cl4_w[:128] @ D_a + 0.25 * cl4_w[128:] @ (sum_j D_j - D_a)
    bias' = dec_b + cl4_b @ sum_j D_j,      D_a = dec_w[128a:128a+128]
This removes ~35% of the FLOPs, the cross-agent reduction, and the
concat entirely.

Sigmoid is rewritten as tanh so every ScalarE function used (tanh,
relu, exp) lives in one activation-table set:
    sigmoid(x) = 0.5 + 0.5 tanh(x/2)
    H0 := tanh(0.5 x + 0.5 enc_b);  fc1 folded: W1' = 0.5 W1,
    b1' = fc1_b + 0.5 colsum(fc1_w)

Layout: pure data parallelism over 8 cores (8192 samples each). All
activations live transposed in SBUF as [feature(=partition), column],
columns agent-planar per 1024-sample super-tile; sample s of agent a
sits at column (s//512)*2048 + a*512 + s%512. The input is
pre-transposed on the host (no on-device transpose), packed two
samples per column ([128, 2048] per super-tile) so the input DMA uses
all 128 partitions and the K=64 enc matmul runs as two concurrent
row-group-tiled matmuls. Trunk matmuls use float32r (full fp32
storage, 1 cycle/row PE path, measured ~1e-4 rel err end-to-end).
The readout runs activation-stationary (lhsT = H3 chunk in bf16 so
fast-weight-load applies), producing logits in natural [sample, class]
orientation, so softmax is a plain free-dim reduction. ScalarE uses
only {tanh, relu, exp} = one activation-table set (sigmoid was
rewritten as tanh with the affine folded into fc1's weights).
"""

import numpy as np

import concourse.bass as bass
import concourse.mybir as mybir
import concourse.tile as tile
from concourse import bacc
from concourse.bass import ts
from concourse.bass_utils import run_bass_kernel_spmd

# ---- problem constants (hardcoded per the task contract) ----
B = 65536
A = 4
OBS = 64
D = 128
C = 16
NCORES = 8
BLOC = B // NCORES          # samples per core
ST = 1024                   # samples per super-tile
NST = BLOC // ST
COLS = A * ST               # transposed columns per super-tile
NCHUNK = 512                # matmul moving-dim chunk (one f32 PSUM bank)
GROUPS = ST // D            # 128-sample readout chunks per super-tile

F32 = mybir.dt.float32
F32R = mybir.dt.float32r    # full fp32 storage, fast PE path
BF16 = mybir.dt.bfloat16
AFT = mybir.ActivationFunctionType
ALU = mybir.AluOpType

# matmul input dtype for the main trunk: F32R (fast, ~fp32 storage) or
# F32 (4x slower PE, bit-accurate) or BF16.
TRUNK_DT = F32R

_compiled = {}


def _build_bass(repeats=1):
    # Bacc (not plain Bass): its compile() runs generate_event_semaphores /
    # move_matmul_waits_to_ldweights, which legalize multi-wait instructions
    # down to the TRN2 limit of one sync wait per instruction.
    # repeats>1 re-emits the whole pipeline (incl. const loads) that many
    # times in one NEFF — used by test.py to measure marginal per-pass
    # device time without the ~85ms axon dispatch round-trip.
    nc = bacc.Bacc()

    # Input packed two-samples-per-column: partitions 0-63 hold features of
    # the first half of each super-tile's samples, 64-127 the second half.
    # Full 128-partition DMA + the enc matmul runs as two concurrent
    # row-group-tiled K=64 matmuls (tile_position (0,0) / (64,0)).
    ot_d = nc.dram_tensor("ot", [2 * OBS, NST, COLS // 2], TRUNK_DT, kind="ExternalInput")
    ew_d = nc.dram_tensor("enc_w", [2 * OBS, D], TRUNK_DT, kind="ExternalInput")
    w1_d = nc.dram_tensor("w1", [D, D], TRUNK_DT, kind="ExternalInput")
    w2_d = nc.dram_tensor("w2", [D, D], TRUNK_DT, kind="ExternalInput")
    w3_d = nc.dram_tensor("w3", [D, D], TRUNK_DT, kind="ExternalInput")
    wz_d = nc.dram_tensor("wz", [D, A * C], BF16, kind="ExternalInput")
    # class bias folded into softmax as exp(bias): probs = e^l e^b / sum
    # (replicated per partition). Pool is nearly idle, so the extra multiply
    # there is free and PE drops its 8 rank-1 bias matmuls per super-tile.
    eb_d = nc.dram_tensor("eb", [D, C], F32, kind="ExternalInput")
    b0_d = nc.dram_tensor("b0", [D, 1], F32, kind="ExternalInput")
    b1_d = nc.dram_tensor("b1", [D, 1], F32, kind="ExternalInput")
    b2_d = nc.dram_tensor("b2", [D, 1], F32, kind="ExternalInput")
    b3_d = nc.dram_tensor("b3", [D, 1], F32, kind="ExternalInput")
    # Output layout [feature-partition, st, group, class] so the store is one
    # DMA of 128 contiguous 32KB per-partition runs (128 descriptors) instead
    # of 8 scatters of 1024 64B descriptors (HW DGE descriptor-gen dominated
    # the measured device time). Host reorders to [BLOC, C] afterwards.
    out_d = nc.dram_tensor("probs", [D, NST * GROUPS * C], F32, kind="ExternalOutput")

    with tile.TileContext(nc) as tc:
        with (
            tc.tile_pool(name="consts", bufs=1) as cpool,
            tc.tile_pool(name="ot", bufs=2) as opool,
            tc.tile_pool(name="acts", bufs=2) as hpool,
            tc.tile_pool(name="soft", bufs=2) as spool,
            tc.tile_pool(name="osb", bufs=2) as obpool,
            tc.tile_pool(name="mmA", bufs=2, space="PSUM") as mmpoolA,
            tc.tile_pool(name="mmD", bufs=2, space="PSUM") as mmpoolD,
            tc.tile_pool(name="lg", bufs=2, space="PSUM") as lgpool,
        ):
            # ot(0) goes FIRST on the SP DMA queue: every const behind it
            # would otherwise delay the first enc matmul by ~1us of DGE
            # dispatch each (measured 12us fill stall). enc deps (ew, b0)
            # follow immediately; deeper-layer weights land later, which is
            # fine because their consumers start later anyway.
            ot0_t = opool.tile([2 * OBS, COLS // 2], TRUNK_DT, tag="ot",
                               name="ot0")
            nc.sync.dma_start(ot0_t[:], ot_d[:, 0, :])
            ew_t = cpool.tile([2 * OBS, D], TRUNK_DT, name="ew")
            nc.sync.dma_start(ew_t[:], ew_d[:])
            b_t = {}
            for nm, dd in (("b0", b0_d), ("b1", b1_d), ("b2", b2_d), ("b3", b3_d)):
                b_t[nm] = cpool.tile([D, 1], F32, name=nm)
                nc.sync.dma_start(b_t[nm][:], dd[:])
            w_t = {}
            for nm, dd in (("w1", w1_d), ("w2", w2_d), ("w3", w3_d)):
                w_t[nm] = cpool.tile([D, D], TRUNK_DT, name=nm)
                nc.sync.dma_start(w_t[nm][:], dd[:])
            wz_t = cpool.tile([D, A * C], BF16, name="wz")
            nc.sync.dma_start(wz_t[:], wz_d[:])
            eb_t = cpool.tile([D, C], F32, name="eb")
            nc.sync.dma_start(eb_t[:], eb_d[:])

            # PSUM can only be read by ACT and DVE (GPSIMD/Pool and the DMA
            # engines are locked out by the BIR verifier), so all post-matmul
            # relu flows through those two. ACT takes 1024-col blocks (its
            # 143ns PSUM-access overhead amortizes), DVE 512-col chunks.
            # Per ST: ACT = 4 enc blocks + 5 fc blocks ~= 9.6us, DVE = 14 fc
            # chunks ~= 10.1us. Pool (GPSIMD, SBUF-only) takes the softmax
            # reduce and normalize multiply.
            # "A" entries below are 1024-col blocks (2 matmul chunks), "D"
            # entries 512-col chunks.
            SCHED = {
                "fc1": ("D",) * 8,
                "fc2": ("A2", "D", "D", "A2", "D", "D"),
                "fc3": ("A2", "D", "D", "A2", "A2"),
            }

            def emit_tail(st, h3, out_sb):
                """Readout + softmax for a finished super-tile.

                Emitted at the TOP of the next iteration (software pipeline):
                its 40 tiny readout matmuls depend only on old data, so the
                in-order PE queue drains them instantly instead of stalling
                the next super-tile's enc matmuls behind the slow fc3 tail.
                Probs land in out_sb (one persistent SBUF tile); a single
                contiguous DMA stores everything after the last super-tile.
                """
                lg = lgpool.tile([D, GROUPS * C], F32, tag="lg")
                for g in range(GROUPS):
                    cbase = (g // 4) * 2048 + (g % 4) * D
                    for a in range(A):
                        nc.tensor.matmul(
                            lg[:, ts(g, C)],
                            h3[:, cbase + a * 512 : cbase + a * 512 + D],
                            wz_t[:, ts(a, C)],
                            start=(a == 0), stop=(a == A - 1),
                        )
                e = spool.tile([D, GROUPS * C], F32, tag="e")
                nc.scalar.activation(e[:], lg[:], AFT.Exp)
                # fold the class bias in as exp(b) on the idle Pool engine
                e2 = spool.tile([D, GROUPS * C], F32, tag="e2")
                nc.gpsimd.tensor_mul(
                    e2[:].rearrange("p (g c) -> p g c", c=C),
                    e[:].rearrange("p (g c) -> p g c", c=C),
                    eb_t[:].unsqueeze(1).broadcast_to([D, GROUPS, C]),
                )
                s = spool.tile([D, GROUPS], F32, tag="s")
                nc.vector.reduce_sum(
                    s[:], e2[:].rearrange("p (g c) -> p g c", c=C),
                    axis=mybir.AxisListType.X,
                )
                r = spool.tile([D, GROUPS], F32, tag="r")
                nc.vector.reciprocal(r[:], s[:])
                nc.gpsimd.tensor_mul(
                    out_sb[:, st * GROUPS * C : (st + 1) * GROUPS * C]
                    .rearrange("p (g c) -> p g c", c=C),
                    e2[:].rearrange("p (g c) -> p g c", c=C),
                    r[:].unsqueeze(2).broadcast_to([D, GROUPS, C]),
                )

            def emit_enc(st, ot_t, h0):
                # enc: tanh(0.5 x + 0.5 b). block j: partition-half hh=j//2
                # of the packed input, 1024 input cols at (j%2)*1024 ->
                # h0 cols j*1024. ACT processes 1024-col blocks.
                for j in range(4):
                    hh = j // 2
                    base = (j % 2) * 1024
                    ps = mmpoolA.tile([D, 1024], F32, tag="mm")
                    for k in range(2):
                        nc.tensor.matmul(
                            ps[:, ts(k, NCHUNK)],
                            ew_t[64 * hh : 64 * (hh + 1), :],
                            ot_t[64 * hh : 64 * (hh + 1),
                                 base + k * NCHUNK : base + (k + 1) * NCHUNK],
                            start=True, stop=True,
                        )
                    nc.scalar.activation(
                        h0[:, ts(j, 1024)], ps[:], AFT.Tanh,
                        bias=b_t["b0"][:], scale=0.5,
                    )

            def emit_fc(layer, src, dst):
                wname = "w%d" % layer
                bname = "b%d" % layer
                col = 0
                for ent in SCHED["fc%d" % layer]:
                    if ent == "A2":
                        ps = mmpoolA.tile([D, 1024], F32, tag="mm")
                        for k in range(2):
                            nc.tensor.matmul(
                                ps[:, ts(k, NCHUNK)],
                                w_t[wname][:],
                                src[:, col + k * NCHUNK : col + (k + 1) * NCHUNK],
                                start=True, stop=True,
                            )
                        nc.scalar.activation(
                            dst[:, col : col + 1024], ps[:], AFT.Relu,
                            bias=b_t[bname][:],
                        )
                        col += 1024
                    else:
                        ps = mmpoolD.tile([D, NCHUNK], F32, tag="mm")
                        nc.tensor.matmul(
                            ps[:],
                            w_t[wname][:],
                            src[:, col : col + NCHUNK],
                            start=True, stop=True,
                        )
                        nc.vector.tensor_scalar(
                            dst[:, col : col + NCHUNK], ps[:],
                            b_t[bname][:], 0.0, ALU.add, ALU.max,
                        )
                        col += NCHUNK
                assert col == COLS

            # ---- 4-deep skewed software pipeline over super-tiles ----
            # Iteration i emits: enc(i), tail(i-3), fc2(i-1), fc3(i-2),
            # fc1(i). Every engine is in-order, so ops must be emitted in
            # the order they become READY: layer L of super-tile st is only
            # emitted one iteration after layer L-1 of st, which means the
            # ACT/DVE/Pool chunks of st's deep layers never block the next
            # super-tile's enc/fc1 stream (the baseline's serialization).
            # ---- continuous 4-deep skewed software pipeline over all
            # repeats * NST super-tiles.  With repeats=1 this is a plain
            # single pass; with repeats>1 consecutive passes overlap exactly
            # like back-to-back kernel invocations with resident weights,
            # which is what the marginal-repeat timing measures. ----
            TOT = repeats * NST
            ot_tiles, h0s, h1s, h2s, h3s = {}, {}, {}, {}, {}
            ot_tiles[0] = ot0_t
            out_sbs = {}

            for i in range(TOT + 3):
                # prefetch first so nothing delays the next input tile
                if i + 1 < TOT:
                    ot_tiles[i + 1] = opool.tile(
                        [2 * OBS, COLS // 2], TRUNK_DT, tag="ot",
                        name="ot%d" % (i + 1),
                    )
                    nc.sync.dma_start(ot_tiles[i + 1][:],
                                      ot_d[:, (i + 1) % NST, :])
                if i < TOT:
                    if i % NST == 0:
                        out_sbs[i // NST] = obpool.tile(
                            [D, NST * GROUPS * C], F32, tag="osb",
                            name="osb%d" % (i // NST),
                        )
                    h0s[i] = hpool.tile([D, COLS], TRUNK_DT, tag="h0",
                                        name="h0_%d" % i)
                    emit_enc(i, ot_tiles.pop(i), h0s[i])
                if i >= 3:
                    st = i - 3
                    emit_tail(st % NST, h3s.pop(st), out_sbs[st // NST])
                    if st % NST == NST - 1:
                        # contiguous store (128 x 32KB descriptors) on the
                        # ACT HWDGE queue so it never head-of-line blocks
                        # the next pass's input loads on the SP queue
                        nc.scalar.dma_start(
                            out_d[:], out_sbs.pop(st // NST)[:])
                if 1 <= i <= TOT:
                    h2s[i - 1] = hpool.tile([D, COLS], TRUNK_DT, tag="h2",
                                            name="h2_%d" % (i - 1))
                    emit_fc(2, h1s.pop(i - 1), h2s[i - 1])
                if 2 <= i <= TOT + 1:
                    h3s[i - 2] = hpool.tile([D, COLS], BF16, tag="h3",
                                            name="h3_%d" % (i - 2))
                    emit_fc(3, h2s.pop(i - 2), h3s[i - 2])
                if i < TOT:
                    h1s[i] = hpool.tile([D, COLS], TRUNK_DT, tag="h1",
                                        name="h1_%d" % i)
                    emit_fc(1, h0s.pop(i), h1s[i])

    nc.compile()
    return nc


def _prep_inputs(inputs):
    """Host-side: fused weights + per-core transposed input shards."""
    f64 = lambda x: np.asarray(x, np.float64)
    enc_w, enc_b = f64(inputs["enc_w"]), f64(inputs["enc_b"])
    fc1_w, fc1_b = f64(inputs["fc1_w"]), f64(inputs["fc1_b"])
    fc2_w, fc2_b = f64(inputs["fc2_w"]), f64(inputs["fc2_b"])
    fc3_w, fc3_b = f64(inputs["fc3_w"]), f64(inputs["fc3_b"])
    cl4_w, cl4_b = f64(inputs["cl4_w"]), f64(inputs["cl4_b"])
    dec_w, dec_b = f64(inputs["dec_w"]), f64(inputs["dec_b"])

    A_ = cl4_w[:D]
    Bm = cl4_w[D:]
    Da = dec_w.reshape(A, D, C)
    Dsum = Da.sum(0)
    Wz = np.concatenate(
        [A_ @ Da[a] + 0.25 * (Bm @ (Dsum - Da[a])) for a in range(A)], axis=1
    )  # [128, 64]
    bias_p = dec_b + cl4_b @ Dsum  # [16]

    import ml_dtypes

    # exp(class bias), replicated per partition — folded into softmax
    eb = np.tile(np.exp(bias_p).astype(np.float32), (D, 1))

    common = {
        "enc_w": np.ascontiguousarray(np.vstack([enc_w, enc_w]), np.float32),
        "w1": np.ascontiguousarray(0.5 * fc1_w, np.float32),
        "w2": np.ascontiguousarray(fc2_w, np.float32),
        "w3": np.ascontiguousarray(fc3_w, np.float32),
        "wz": np.ascontiguousarray(Wz).astype(ml_dtypes.bfloat16),
        "eb": eb,
        "b0": (0.5 * enc_b).astype(np.float32).reshape(D, 1),
        "b1": (fc1_b + 0.5 * fc1_w.sum(0)).astype(np.float32).reshape(D, 1),
        "b2": fc2_b.astype(np.float32).reshape(D, 1),
        "b3": fc3_b.astype(np.float32).reshape(D, 1),
    }

    O = np.asarray(inputs["O"], np.float32)  # [B, A, OBS]
    in_maps = []
    for c in range(NCORES):
        oc = O[c * BLOC : (c + 1) * BLOC]                  # [BLOC, A, OBS]
        # ot[h*64+f, st, a*512+s'] = O[st*1024 + h*512 + s', a, f]
        x = oc.reshape(NST, 2, ST // 2, A, OBS)
        ot = np.ascontiguousarray(x.transpose(1, 4, 0, 3, 2)).reshape(
            2 * OBS, NST, COLS // 2
        )
        in_maps.append({"ot": ot, **common})
    return in_maps


def build(repeats=1):
    key = "nc%d" % repeats
    if key not in _compiled:
        _compiled[key] = _build_bass(repeats)
    return _compiled[key]


def kernel(**inputs):
    nc = build(1)
    in_maps = _prep_inputs(inputs)
    res = run_bass_kernel_spmd(nc, in_maps, core_ids=list(range(NCORES)))
    # device layout [p, st*G*C]: sample (st*1024 + g*128 + p), class c sits
    # at [p, st*128 + g*16 + c] — undo on host
    outs = [
        res.results[i]["probs"]
        .reshape(D, NST, GROUPS, C)
        .transpose(1, 2, 0, 3)
        .reshape(BLOC, C)
        for i in range(NCORES)
    ]
    return np.ascontiguousarray(np.concatenate(outs, axis=0))



# revision 24
# speedup vs baseline: 1.3319x; 1.3319x over previous
"""Trainium2 Bass kernel for nn_CommNetActor.

Network (per sample, 4 agents, all weights shared across agents):
    H0 = sigmoid(O @ enc_w + enc_b)            [B,4,128]
    H1..H3 = relu chain of 128x128 fc layers
    C  = (sum_j H3[:,j] - H3) / 4              (CommNet neighbour mean)
    H4 = [H3 | C] @ cl4_w + cl4_b
    logits = H4.reshape(B,512) @ dec_w + dec_b
    out = softmax(logits)                      [B,16]

Key algebraic fold used here: since C is linear in the H3 agent slices,
the whole tail (neighbour mean + cl4 + dec) collapses into per-agent
readout matrices applied directly to H3:
    logits[b] = sum_a H3[b,a] @ Wz_a + bias'
    Wz_a  = cl4_w[:128] @ D_a + 0.25 * cl4_w[128:] @ (sum_j D_j - D_a)
    bias' = dec_b + cl4_b @ sum_j D_j,      D_a = dec_w[128a:128a+128]
This removes ~35% of the FLOPs, the cross-agent reduction, and the
concat entirely.

Sigmoid is rewritten as tanh so every ScalarE function used (tanh,
relu, exp) lives in one activation-table set:
    sigmoid(x) = 0.5 + 0.5 tanh(x/2)
    H0 := tanh(0.5 x + 0.5 enc_b);  fc1 folded: W1' = 0.5 W1,
    b1' = fc1_b + 0.5 colsum(fc1_w)

Layout: pure data parallelism over 8 cores (8192 samples each). All
activations live transposed in SBUF as [feature(=partition), column],
columns agent-planar per 1024-sample super-tile; sample s of agent a
sits at column (s//512)*2048 + a*512 + s%512. The input is
pre-transposed on the host (no on-device transpose), packed two
samples per column ([128, 2048] per super-tile) so the input DMA uses
all 128 partitions and the K=64 enc matmul runs as two concurrent
row-group-tiled matmuls. Trunk matmuls use float32r (full fp32
storage, 1 cycle/row PE path, measured ~1e-4 rel err end-to-end).
The readout runs activation-stationary (lhsT = H3 chunk in bf16 so
fast-weight-load applies), producing logits in natural [sample, class]
orientation, so softmax is a plain free-dim reduction. ScalarE uses
only {tanh, relu, exp} = one activation-table set (sigmoid was
rewritten as tanh with the affine folded into fc1's weights).
"""

import numpy as np

import concourse.bass as bass
import concourse.mybir as mybir
import concourse.tile as tile
from concourse import bacc
from concourse.bass import ts
from concourse.bass_utils import run_bass_kernel_spmd

# ---- problem constants (hardcoded per the task contract) ----
B = 65536
A = 4
OBS = 64
D = 128
C = 16
NCORES = 8
BLOC = B // NCORES          # samples per core
ST = 1024                   # samples per super-tile
NST = BLOC // ST
COLS = A * ST               # transposed columns per super-tile
NCHUNK = 512                # matmul moving-dim chunk (one f32 PSUM bank)
GROUPS = ST // D            # 128-sample readout chunks per super-tile

F32 = mybir.dt.float32
F32R = mybir.dt.float32r    # full fp32 storage, fast PE path
BF16 = mybir.dt.bfloat16
AFT = mybir.ActivationFunctionType
ALU = mybir.AluOpType

# matmul input dtype for the main trunk: F32R (fast, ~fp32 storage) or
# F32 (4x slower PE, bit-accurate) or BF16.
TRUNK_DT = F32R

_compiled = {}


def _build_bass(repeats=1):
    # Bacc (not plain Bass): its compile() runs generate_event_semaphores /
    # move_matmul_waits_to_ldweights, which legalize multi-wait instructions
    # down to the TRN2 limit of one sync wait per instruction.
    # repeats>1 re-emits the whole pipeline (incl. const loads) that many
    # times in one NEFF — used by test.py to measure marginal per-pass
    # device time without the ~85ms axon dispatch round-trip.
    nc = bacc.Bacc()

    # Input packed two-samples-per-column: partitions 0-63 hold features of
    # the first half of each super-tile's samples, 64-127 the second half.
    # Full 128-partition DMA + the enc matmul runs as two concurrent
    # row-group-tiled K=64 matmuls (tile_position (0,0) / (64,0)).
    ot_d = nc.dram_tensor("ot", [2 * OBS, NST, COLS // 2], TRUNK_DT, kind="ExternalInput")
    ew_d = nc.dram_tensor("enc_w", [2 * OBS, D], TRUNK_DT, kind="ExternalInput")
    w1_d = nc.dram_tensor("w1", [D, D], TRUNK_DT, kind="ExternalInput")
    w2_d = nc.dram_tensor("w2", [D, D], TRUNK_DT, kind="ExternalInput")
    w3_d = nc.dram_tensor("w3", [D, D], TRUNK_DT, kind="ExternalInput")
    wz_d = nc.dram_tensor("wz", [D, A * C], BF16, kind="ExternalInput")
    # class bias folded into softmax as exp(bias): probs = e^l e^b / sum
    # (replicated per partition). Pool is nearly idle, so the extra multiply
    # there is free and PE drops its 8 rank-1 bias matmuls per super-tile.
    eb_d = nc.dram_tensor("eb", [D, C], F32, kind="ExternalInput")
    b0_d = nc.dram_tensor("b0", [D, 1], F32, kind="ExternalInput")
    b1_d = nc.dram_tensor("b1", [D, 1], F32, kind="ExternalInput")
    b2_d = nc.dram_tensor("b2", [D, 1], F32, kind="ExternalInput")
    b3_d = nc.dram_tensor("b3", [D, 1], F32, kind="ExternalInput")
    # Output layout [feature-partition, st, group, class] so the store is one
    # DMA of 128 contiguous 32KB per-partition runs (128 descriptors) instead
    # of 8 scatters of 1024 64B descriptors (HW DGE descriptor-gen dominated
    # the measured device time). Host reorders to [BLOC, C] afterwards.
    out_d = nc.dram_tensor("probs", [D, NST * GROUPS * C], F32, kind="ExternalOutput")

    with tile.TileContext(nc) as tc:
        with (
            tc.tile_pool(name="consts", bufs=1) as cpool,
            tc.tile_pool(name="ot", bufs=2) as opool,
            tc.tile_pool(name="acts", bufs=2) as hpool,
            tc.tile_pool(name="soft", bufs=2) as spool,
            tc.tile_pool(name="osb", bufs=2) as obpool,
            tc.tile_pool(name="mmA", bufs=2, space="PSUM") as mmpoolA,
            tc.tile_pool(name="mmD", bufs=3, space="PSUM") as mmpoolD,
            tc.tile_pool(name="lg", bufs=1, space="PSUM") as lgpool,
        ):
            # ot(0) goes FIRST on the SP DMA queue: every const behind it
            # would otherwise delay the first enc matmul by ~1us of DGE
            # dispatch each (measured 12us fill stall). enc deps (ew, b0)
            # follow immediately; deeper-layer weights land later, which is
            # fine because their consumers start later anyway.
            ot0_t = opool.tile([2 * OBS, COLS // 2], TRUNK_DT, tag="ot",
                               name="ot0")
            nc.sync.dma_start(ot0_t[:], ot_d[:, 0, :])
            ew_t = cpool.tile([2 * OBS, D], TRUNK_DT, name="ew")
            nc.sync.dma_start(ew_t[:], ew_d[:])
            b_t = {}
            for nm, dd in (("b0", b0_d), ("b1", b1_d), ("b2", b2_d), ("b3", b3_d)):
                b_t[nm] = cpool.tile([D, 1], F32, name=nm)
                nc.sync.dma_start(b_t[nm][:], dd[:])
            w_t = {}
            for nm, dd in (("w1", w1_d), ("w2", w2_d), ("w3", w3_d)):
                w_t[nm] = cpool.tile([D, D], TRUNK_DT, name=nm)
                nc.sync.dma_start(w_t[nm][:], dd[:])
            wz_t = cpool.tile([D, A * C], BF16, name="wz")
            nc.sync.dma_start(wz_t[:], wz_d[:])
            eb_t = cpool.tile([D, C], F32, name="eb")
            nc.sync.dma_start(eb_t[:], eb_d[:])

            # PSUM can only be read by ACT and DVE (GPSIMD/Pool and the DMA
            # engines are locked out by the BIR verifier — verified: walrus
            # rejects a Pool PSUM read), so all post-matmul relu flows
            # through those two. ACT takes 1024-col blocks (its PSUM-access
            # overhead amortizes), DVE 512-col chunks. The pools are split
            # BY CONSUMER ENGINE: a shared pool lets a run of same-engine
            # blocks head-of-line block the in-order PE (measured +20us in
            # sim). Per ST: ACT = 4 enc + 5 fc blocks + exp ~= 9.6us, DVE =
            # 14 fc chunks + softmax reduce/reciprocal ~= 9.9us. Pool
            # (GPSIMD, SBUF-only) takes the two softmax multiplies.
            # "A2" entries are 1024-col blocks (2 matmul chunks), "D"
            # entries 512-col chunks.
            SCHED = {
                "fc1": ("D",) * 8,
                "fc2": ("A2", "D", "D", "A2", "D", "D"),
                "fc3": ("A2", "D", "D", "A2", "A2"),
            }

            def emit_tail(st, h3, out_sb):
                """Readout + softmax for a finished super-tile.

                Emitted at the TOP of the next iteration (software pipeline):
                its 40 tiny readout matmuls depend only on old data, so the
                in-order PE queue drains them instantly instead of stalling
                the next super-tile's enc matmuls behind the slow fc3 tail.
                Probs land in out_sb (one persistent SBUF tile); a single
                contiguous DMA stores everything after the last super-tile.
                """
                lg = lgpool.tile([D, GROUPS * C], F32, tag="lg")
                for g in range(GROUPS):
                    cbase = (g // 4) * 2048 + (g % 4) * D
                    for a in range(A):
                        nc.tensor.matmul(
                            lg[:, ts(g, C)],
                            h3[:, cbase + a * 512 : cbase + a * 512 + D],
                            wz_t[:, ts(a, C)],
                            start=(a == 0), stop=(a == A - 1),
                        )
                e = spool.tile([D, GROUPS * C], F32, tag="e")
                nc.scalar.activation(e[:], lg[:], AFT.Exp)
                # fold the class bias in as exp(b) on the idle Pool engine
                e2 = spool.tile([D, GROUPS * C], F32, tag="e2")
                nc.gpsimd.tensor_mul(
                    e2[:].rearrange("p (g c) -> p g c", c=C),
                    e[:].rearrange("p (g c) -> p g c", c=C),
                    eb_t[:].unsqueeze(1).broadcast_to([D, GROUPS, C]),
                )
                s = spool.tile([D, GROUPS], F32, tag="s")
                nc.vector.reduce_sum(
                    s[:], e2[:].rearrange("p (g c) -> p g c", c=C),
                    axis=mybir.AxisListType.X,
                )
                r = spool.tile([D, GROUPS], F32, tag="r")
                nc.vector.reciprocal(r[:], s[:])
                nc.gpsimd.tensor_mul(
                    out_sb[:, st * GROUPS * C : (st + 1) * GROUPS * C]
                    .rearrange("p (g c) -> p g c", c=C),
                    e2[:].rearrange("p (g c) -> p g c", c=C),
                    r[:].unsqueeze(2).broadcast_to([D, GROUPS, C]),
                )

            def emit_enc(st, ot_t, h0):
                # enc: tanh(0.5 x + 0.5 b). block j: partition-half hh=j//2
                # of the packed input, 1024 input cols at (j%2)*1024 ->
                # h0 cols j*1024. ACT processes 1024-col blocks.
                for j in range(4):
                    hh = j // 2
                    base = (j % 2) * 1024
                    ps = mmpoolA.tile([D, 1024], F32, tag="mm")
                    for k in range(2):
                        nc.tensor.matmul(
                            ps[:, ts(k, NCHUNK)],
                            ew_t[64 * hh : 64 * (hh + 1), :],
                            ot_t[64 * hh : 64 * (hh + 1),
                                 base + k * NCHUNK : base + (k + 1) * NCHUNK],
                            start=True, stop=True,
                        )
                    nc.scalar.activation(
                        h0[:, ts(j, 1024)], ps[:], AFT.Tanh,
                        bias=b_t["b0"][:], scale=0.5,
                    )

            def emit_fc(layer, src, dst):
                wname = "w%d" % layer
                bname = "b%d" % layer
                col = 0
                for ent in SCHED["fc%d" % layer]:
                    if ent == "A2":
                        ps = mmpoolA.tile([D, 1024], F32, tag="mm")
                        for k in range(2):
                            nc.tensor.matmul(
                                ps[:, ts(k, NCHUNK)],
                                w_t[wname][:],
                                src[:, col + k * NCHUNK : col + (k + 1) * NCHUNK],
                                start=True, stop=True,
                            )
                        nc.scalar.activation(
                            dst[:, col : col + 1024], ps[:], AFT.Relu,
                            bias=b_t[bname][:],
                        )
                        col += 1024
                    else:
                        ps = mmpoolD.tile([D, NCHUNK], F32, tag="mm")
                        nc.tensor.matmul(
                            ps[:],
                            w_t[wname][:],
                            src[:, col : col + NCHUNK],
                            start=True, stop=True,
                        )
                        nc.vector.tensor_scalar(
                            dst[:, col : col + NCHUNK], ps[:],
                            b_t[bname][:], 0.0, ALU.add, ALU.max,
                        )
                        col += NCHUNK
                assert col == COLS

            # ---- 4-deep skewed software pipeline over super-tiles ----
            # Iteration i emits: enc(i), tail(i-3), fc2(i-1), fc3(i-2),
            # fc1(i). Every engine is in-order, so ops must be emitted in
            # the order they become READY: layer L of super-tile st is only
            # emitted one iteration after layer L-1 of st, which means the
            # ACT/DVE/Pool chunks of st's deep layers never block the next
            # super-tile's enc/fc1 stream (the baseline's serialization).
            # ---- continuous 4-deep skewed software pipeline over all
            # repeats * NST super-tiles.  With repeats=1 this is a plain
            # single pass; with repeats>1 consecutive passes overlap exactly
            # like back-to-back kernel invocations with resident weights,
            # which is what the marginal-repeat timing measures. ----
            TOT = repeats * NST
            ot_tiles, h0s, h1s, h2s, h3s = {}, {}, {}, {}, {}
            ot_tiles[0] = ot0_t
            out_sbs = {}

            for i in range(TOT + 3):
                # prefetch first so nothing delays the next input tile
                if i + 1 < TOT:
                    ot_tiles[i + 1] = opool.tile(
                        [2 * OBS, COLS // 2], TRUNK_DT, tag="ot",
                        name="ot%d" % (i + 1),
                    )
                    nc.sync.dma_start(ot_tiles[i + 1][:],
                                      ot_d[:, (i + 1) % NST, :])
                if i < TOT:
                    if i % NST == 0:
                        out_sbs[i // NST] = obpool.tile(
                            [D, NST * GROUPS * C], F32, tag="osb",
                            name="osb%d" % (i // NST),
                        )
                    h0s[i] = hpool.tile([D, COLS], TRUNK_DT, tag="h0",
                                        name="h0_%d" % i)
                    emit_enc(i, ot_tiles.pop(i), h0s[i])
                if i >= 3:
                    st = i - 3
                    emit_tail(st % NST, h3s.pop(st), out_sbs[st // NST])
                    if st % NST == NST - 1:
                        # contiguous store (128 x 32KB descriptors) on the
                        # ACT HWDGE queue so it never head-of-line blocks
                        # the next pass's input loads on the SP queue
                        nc.scalar.dma_start(
                            out_d[:], out_sbs.pop(st // NST)[:])
                if 1 <= i <= TOT:
                    h2s[i - 1] = hpool.tile([D, COLS], TRUNK_DT, tag="h2",
                                            name="h2_%d" % (i - 1))
                    emit_fc(2, h1s.pop(i - 1), h2s[i - 1])
                if 2 <= i <= TOT + 1:
                    h3s[i - 2] = hpool.tile([D, COLS], BF16, tag="h3",
                                            name="h3_%d" % (i - 2))
                    emit_fc(3, h2s.pop(i - 2), h3s[i - 2])
                if i < TOT:
                    h1s[i] = hpool.tile([D, COLS], TRUNK_DT, tag="h1",
                                        name="h1_%d" % i)
                    emit_fc(1, h0s.pop(i), h1s[i])

    nc.compile()
    return nc


def _prep_inputs(inputs):
    """Host-side: fused weights + per-core transposed input shards."""
    f64 = lambda x: np.asarray(x, np.float64)
    enc_w, enc_b = f64(inputs["enc_w"]), f64(inputs["enc_b"])
    fc1_w, fc1_b = f64(inputs["fc1_w"]), f64(inputs["fc1_b"])
    fc2_w, fc2_b = f64(inputs["fc2_w"]), f64(inputs["fc2_b"])
    fc3_w, fc3_b = f64(inputs["fc3_w"]), f64(inputs["fc3_b"])
    cl4_w, cl4_b = f64(inputs["cl4_w"]), f64(inputs["cl4_b"])
    dec_w, dec_b = f64(inputs["dec_w"]), f64(inputs["dec_b"])

    A_ = cl4_w[:D]
    Bm = cl4_w[D:]
    Da = dec_w.reshape(A, D, C)
    Dsum = Da.sum(0)
    Wz = np.concatenate(
        [A_ @ Da[a] + 0.25 * (Bm @ (Dsum - Da[a])) for a in range(A)], axis=1
    )  # [128, 64]
    bias_p = dec_b + cl4_b @ Dsum  # [16]

    import ml_dtypes

    # exp(class bias), replicated per partition — folded into softmax
    eb = np.tile(np.exp(bias_p).astype(np.float32), (D, 1))

    common = {
        "enc_w": np.ascontiguousarray(np.vstack([enc_w, enc_w]), np.float32),
        "w1": np.ascontiguousarray(0.5 * fc1_w, np.float32),
        "w2": np.ascontiguousarray(fc2_w, np.float32),
        "w3": np.ascontiguousarray(fc3_w, np.float32),
        "wz": np.ascontiguousarray(Wz).astype(ml_dtypes.bfloat16),
        "eb": eb,
        "b0": (0.5 * enc_b).astype(np.float32).reshape(D, 1),
        "b1": (fc1_b + 0.5 * fc1_w.sum(0)).astype(np.float32).reshape(D, 1),
        "b2": fc2_b.astype(np.float32).reshape(D, 1),
        "b3": fc3_b.astype(np.float32).reshape(D, 1),
    }

    O = np.asarray(inputs["O"], np.float32)  # [B, A, OBS]
    in_maps = []
    for c in range(NCORES):
        oc = O[c * BLOC : (c + 1) * BLOC]                  # [BLOC, A, OBS]
        # ot[h*64+f, st, a*512+s'] = O[st*1024 + h*512 + s', a, f]
        x = oc.reshape(NST, 2, ST // 2, A, OBS)
        ot = np.ascontiguousarray(x.transpose(1, 4, 0, 3, 2)).reshape(
            2 * OBS, NST, COLS // 2
        )
        in_maps.append({"ot": ot, **common})
    return in_maps


def build(repeats=1):
    key = "nc%d" % repeats
    if key not in _compiled:
        _compiled[key] = _build_bass(repeats)
    return _compiled[key]


def kernel(**inputs):
    nc = build(1)
    in_maps = _prep_inputs(inputs)
    res = run_bass_kernel_spmd(nc, in_maps, core_ids=list(range(NCORES)))
    # device layout [p, st*G*C]: sample (st*1024 + g*128 + p), class c sits
    # at [p, st*128 + g*16 + c] — undo on host
    outs = [
        res.results[i]["probs"]
        .reshape(D, NST, GROUPS, C)
        .transpose(1, 2, 0, 3)
        .reshape(BLOC, C)
        for i in range(NCORES)
    ]
    return np.ascontiguousarray(np.concatenate(outs, axis=0))



# revision 25
# speedup vs baseline: 1.5305x; 1.1491x over previous
"""Trainium2 Bass kernel for nn_CommNetActor.

Network (per sample, 4 agents, all weights shared across agents):
    H0 = sigmoid(O @ enc_w + enc_b)            [B,4,128]
    H1..H3 = relu chain of 128x128 fc layers
    C  = (sum_j H3[:,j] - H3) / 4              (CommNet neighbour mean)
    H4 = [H3 | C] @ cl4_w + cl4_b
    logits = H4.reshape(B,512) @ dec_w + dec_b
    out = softmax(logits)                      [B,16]

Key algebraic fold used here: since C is linear in the H3 agent slices,
the whole tail (neighbour mean + cl4 + dec) collapses into per-agent
readout matrices applied directly to H3:
    logits[b] = sum_a H3[b,a] @ Wz_a + bias'
    Wz_a  = cl4_w[:128] @ D_a + 0.25 * cl4_w[128:] @ (sum_j D_j - D_a)
    bias' = dec_b + cl4_b @ sum_j D_j,      D_a = dec_w[128a:128a+128]
This removes ~35% of the FLOPs, the cross-agent reduction, and the
concat entirely.

Sigmoid is rewritten as tanh so every ScalarE function used (tanh,
relu, exp) lives in one activation-table set:
    sigmoid(x) = 0.5 + 0.5 tanh(x/2)
    H0 := tanh(0.5 x + 0.5 enc_b);  fc1 folded: W1' = 0.5 W1,
    b1' = fc1_b + 0.5 colsum(fc1_w)

Layout: pure data parallelism over 8 cores (8192 samples each). All
activations live transposed in SBUF as [feature(=partition), column],
columns agent-planar per 1024-sample super-tile; sample s of agent a
sits at column (s//512)*2048 + a*512 + s%512. The input is
pre-transposed on the host (no on-device transpose), packed two
samples per column ([128, 2048] per super-tile) so the input DMA uses
all 128 partitions and the K=64 enc matmul runs as two concurrent
row-group-tiled matmuls. Trunk matmuls use float32r (full fp32
storage, 1 cycle/row PE path, measured ~1e-4 rel err end-to-end).
The readout runs activation-stationary (lhsT = H3 chunk in bf16 so
fast-weight-load applies), producing logits in natural [sample, class]
orientation, so softmax is a plain free-dim reduction. The class bias
is folded in as exp(bias') on the GPSIMD/Pool engine inside the
softmax (probs = e^l e^b / sum e^l e^b), removing 8 PE matmuls per
super-tile. ScalarE uses only {tanh, relu, exp} = one activation-table
set (sigmoid was rewritten as tanh with the affine folded into fc1's
weights).

Output leaves the device as [feature-partition, st*group*class] so the
store is one DMA of 128 contiguous 32KB per-partition runs; the 64B
per-sample scatter it replaces cost ~20ns/descriptor of real HWDGE
descriptor-generation (8 x 1024 descriptors ~ 160us/pass, the dominant
term of the previous kernel). kernel() reorders rows on the host.
"""

import numpy as np

import concourse.bass as bass
import concourse.mybir as mybir
import concourse.tile as tile
from concourse import bacc
from concourse.bass import ts
from concourse.bass_utils import run_bass_kernel_spmd

# ---- problem constants (hardcoded per the task contract) ----
B = 65536
A = 4
OBS = 64
D = 128
C = 16
NCORES = 8
BLOC = B // NCORES          # samples per core
ST = 1024                   # samples per super-tile
NST = BLOC // ST
COLS = A * ST               # transposed columns per super-tile
NCHUNK = 512                # matmul moving-dim chunk (one f32 PSUM bank)
GROUPS = ST // D            # 128-sample readout chunks per super-tile

F32 = mybir.dt.float32
F32R = mybir.dt.float32r    # full fp32 storage, fast PE path
BF16 = mybir.dt.bfloat16
AFT = mybir.ActivationFunctionType
ALU = mybir.AluOpType

# matmul input dtype for the main trunk: F32R (fast, ~fp32 storage) or
# F32 (4x slower PE, bit-accurate) or BF16.
TRUNK_DT = F32R

_compiled = {}


def _build_bass(repeats=1):
    # Bacc (not plain Bass): its compile() runs generate_event_semaphores /
    # move_matmul_waits_to_ldweights, which legalize multi-wait instructions
    # down to the TRN2 limit of one sync wait per instruction.
    # repeats>1 re-emits the whole pipeline (incl. const loads) that many
    # times in one NEFF — used by test.py to measure marginal per-pass
    # device time without the ~85ms axon dispatch round-trip.
    nc = bacc.Bacc()

    # Input packed two-samples-per-column: partitions 0-63 hold features of
    # the first half of each super-tile's samples, 64-127 the second half.
    # Full 128-partition DMA + the enc matmul runs as two concurrent
    # row-group-tiled K=64 matmuls (tile_position (0,0) / (64,0)).
    ot_d = nc.dram_tensor("ot", [2 * OBS, NST, COLS // 2], TRUNK_DT, kind="ExternalInput")
    ew_d = nc.dram_tensor("enc_w", [2 * OBS, D], TRUNK_DT, kind="ExternalInput")
    w1_d = nc.dram_tensor("w1", [D, D], TRUNK_DT, kind="ExternalInput")
    w2_d = nc.dram_tensor("w2", [D, D], TRUNK_DT, kind="ExternalInput")
    w3_d = nc.dram_tensor("w3", [D, D], TRUNK_DT, kind="ExternalInput")
    wz_d = nc.dram_tensor("wz", [D, A * C], BF16, kind="ExternalInput")
    # class bias folded into softmax as exp(bias): probs = e^l e^b / sum
    # (replicated per partition). Pool is nearly idle, so the extra multiply
    # there is free and PE drops its 8 rank-1 bias matmuls per super-tile.
    eb_d = nc.dram_tensor("eb", [D, C], F32, kind="ExternalInput")
    b0_d = nc.dram_tensor("b0", [D, 1], F32, kind="ExternalInput")
    b1_d = nc.dram_tensor("b1", [D, 1], F32, kind="ExternalInput")
    b2_d = nc.dram_tensor("b2", [D, 1], F32, kind="ExternalInput")
    b3_d = nc.dram_tensor("b3", [D, 1], F32, kind="ExternalInput")
    # Output layout [feature-partition, st, group, class] so the store is one
    # DMA of 128 contiguous 32KB per-partition runs (128 descriptors) instead
    # of 8 scatters of 1024 64B descriptors (HW DGE descriptor-gen dominated
    # the measured device time). Host reorders to [BLOC, C] afterwards.
    out_d = nc.dram_tensor("probs", [D, NST * GROUPS * C], F32, kind="ExternalOutput")

    with tile.TileContext(nc) as tc:
        with (
            tc.tile_pool(name="consts", bufs=1) as cpool,
            tc.tile_pool(name="ot", bufs=2) as opool,
            tc.tile_pool(name="acts", bufs=2) as hpool,
            tc.tile_pool(name="soft", bufs=2) as spool,
            tc.tile_pool(name="osb", bufs=2) as obpool,
            tc.tile_pool(name="mmA", bufs=2, space="PSUM") as mmpoolA,
            tc.tile_pool(name="mmD", bufs=3, space="PSUM") as mmpoolD,
            tc.tile_pool(name="lg", bufs=1, space="PSUM") as lgpool,
        ):
            # ot(0) goes FIRST on the SP DMA queue: every const behind it
            # would otherwise delay the first enc matmul by ~1us of DGE
            # dispatch each (measured 12us fill stall). enc deps (ew, b0)
            # follow immediately; deeper-layer weights land later, which is
            # fine because their consumers start later anyway.
            ot0_t = opool.tile([2 * OBS, COLS // 2], TRUNK_DT, tag="ot",
                               name="ot0")
            nc.sync.dma_start(ot0_t[:], ot_d[:, 0, :])
            ew_t = cpool.tile([2 * OBS, D], TRUNK_DT, name="ew")
            nc.sync.dma_start(ew_t[:], ew_d[:])
            b_t = {}
            for nm, dd in (("b0", b0_d), ("b1", b1_d), ("b2", b2_d), ("b3", b3_d)):
                b_t[nm] = cpool.tile([D, 1], F32, name=nm)
                nc.sync.dma_start(b_t[nm][:], dd[:])
            w_t = {}
            for nm, dd in (("w1", w1_d), ("w2", w2_d), ("w3", w3_d)):
                w_t[nm] = cpool.tile([D, D], TRUNK_DT, name=nm)
                nc.sync.dma_start(w_t[nm][:], dd[:])
            wz_t = cpool.tile([D, A * C], BF16, name="wz")
            nc.sync.dma_start(wz_t[:], wz_d[:])
            eb_t = cpool.tile([D, C], F32, name="eb")
            nc.sync.dma_start(eb_t[:], eb_d[:])

            # PSUM can only be read by ACT and DVE (GPSIMD/Pool and the DMA
            # engines are locked out by the BIR verifier — verified: walrus
            # rejects a Pool PSUM read), so all post-matmul relu flows
            # through those two. ACT takes 1024-col blocks (its PSUM-access
            # overhead amortizes), DVE 512-col chunks. The pools are split
            # BY CONSUMER ENGINE: a shared pool lets a run of same-engine
            # blocks head-of-line block the in-order PE (measured +20us in
            # sim). Per ST: ACT = 4 enc + 5 fc blocks + exp ~= 9.6us, DVE =
            # 14 fc chunks + softmax reduce/reciprocal ~= 9.9us. Pool
            # (GPSIMD, SBUF-only) takes the two softmax multiplies.
            # "A2" entries are 1024-col blocks (2 matmul chunks), "D"
            # entries 512-col chunks.
            SCHED = {
                "fc1": ("D",) * 8,
                "fc2": ("A2", "D", "D", "A2", "D", "D"),
                "fc3": ("A2", "D", "D", "A2", "A2"),
            }

            def emit_tail(st, h3, out_sb):
                """Readout + softmax for a finished super-tile.

                Emitted at the TOP of the next iteration (software pipeline):
                its 40 tiny readout matmuls depend only on old data, so the
                in-order PE queue drains them instantly instead of stalling
                the next super-tile's enc matmuls behind the slow fc3 tail.
                Probs land in out_sb (one persistent SBUF tile); a single
                contiguous DMA stores everything after the last super-tile.
                """
                lg = lgpool.tile([D, GROUPS * C], F32, tag="lg")
                for g in range(GROUPS):
                    cbase = (g // 4) * 2048 + (g % 4) * D
                    for a in range(A):
                        nc.tensor.matmul(
                            lg[:, ts(g, C)],
                            h3[:, cbase + a * 512 : cbase + a * 512 + D],
                            wz_t[:, ts(a, C)],
                            start=(a == 0), stop=(a == A - 1),
                        )
                e = spool.tile([D, GROUPS * C], F32, tag="e")
                nc.scalar.activation(e[:], lg[:], AFT.Exp)
                # fold the class bias in as exp(b) on the idle Pool engine
                e2 = spool.tile([D, GROUPS * C], F32, tag="e2")
                nc.gpsimd.tensor_mul(
                    e2[:].rearrange("p (g c) -> p g c", c=C),
                    e[:].rearrange("p (g c) -> p g c", c=C),
                    eb_t[:].unsqueeze(1).broadcast_to([D, GROUPS, C]),
                )
                s = spool.tile([D, GROUPS], F32, tag="s")
                nc.vector.reduce_sum(
                    s[:], e2[:].rearrange("p (g c) -> p g c", c=C),
                    axis=mybir.AxisListType.X,
                )
                r = spool.tile([D, GROUPS], F32, tag="r")
                nc.vector.reciprocal(r[:], s[:])
                nc.gpsimd.tensor_mul(
                    out_sb[:, st * GROUPS * C : (st + 1) * GROUPS * C]
                    .rearrange("p (g c) -> p g c", c=C),
                    e2[:].rearrange("p (g c) -> p g c", c=C),
                    r[:].unsqueeze(2).broadcast_to([D, GROUPS, C]),
                )

            def emit_enc(st, ot_t, h0):
                # enc: tanh(0.5 x + 0.5 b). block j: partition-half hh=j//2
                # of the packed input, 1024 input cols at (j%2)*1024 ->
                # h0 cols j*1024. ACT processes 1024-col blocks.
                for j in range(4):
                    hh = j // 2
                    base = (j % 2) * 1024
                    ps = mmpoolA.tile([D, 1024], F32, tag="mm")
                    for k in range(2):
                        nc.tensor.matmul(
                            ps[:, ts(k, NCHUNK)],
                            ew_t[64 * hh : 64 * (hh + 1), :],
                            ot_t[64 * hh : 64 * (hh + 1),
                                 base + k * NCHUNK : base + (k + 1) * NCHUNK],
                            start=True, stop=True,
                        )
                    nc.scalar.activation(
                        h0[:, ts(j, 1024)], ps[:], AFT.Tanh,
                        bias=b_t["b0"][:], scale=0.5,
                    )

            def emit_fc(layer, src, dst):
                wname = "w%d" % layer
                bname = "b%d" % layer
                col = 0
                for ent in SCHED["fc%d" % layer]:
                    if ent == "A2":
                        ps = mmpoolA.tile([D, 1024], F32, tag="mm")
                        for k in range(2):
                            nc.tensor.matmul(
                                ps[:, ts(k, NCHUNK)],
                                w_t[wname][:],
                                src[:, col + k * NCHUNK : col + (k + 1) * NCHUNK],
                                start=True, stop=True,
                            )
                        nc.scalar.activation(
                            dst[:, col : col + 1024], ps[:], AFT.Relu,
                            bias=b_t[bname][:],
                        )
                        col += 1024
                    else:
                        ps = mmpoolD.tile([D, NCHUNK], F32, tag="mm")
                        nc.tensor.matmul(
                            ps[:],
                            w_t[wname][:],
                            src[:, col : col + NCHUNK],
                            start=True, stop=True,
                        )
                        nc.vector.tensor_scalar(
                            dst[:, col : col + NCHUNK], ps[:],
                            b_t[bname][:], 0.0, ALU.add, ALU.max,
                        )
                        col += NCHUNK
                assert col == COLS

            # ---- 4-deep skewed software pipeline over super-tiles ----
            # Iteration i emits: enc(i), tail(i-3), fc2(i-1), fc3(i-2),
            # fc1(i). Every engine is in-order, so ops must be emitted in
            # the order they become READY: layer L of super-tile st is only
            # emitted one iteration after layer L-1 of st, which means the
            # ACT/DVE/Pool chunks of st's deep layers never block the next
            # super-tile's enc/fc1 stream (the baseline's serialization).
            # ---- continuous 4-deep skewed software pipeline over all
            # repeats * NST super-tiles.  With repeats=1 this is a plain
            # single pass; with repeats>1 consecutive passes overlap exactly
            # like back-to-back kernel invocations with resident weights,
            # which is what the marginal-repeat timing measures. ----
            TOT = repeats * NST
            ot_tiles, h0s, h1s, h2s, h3s = {}, {}, {}, {}, {}
            ot_tiles[0] = ot0_t
            out_sbs = {}

            for i in range(TOT + 3):
                # prefetch first so nothing delays the next input tile
                if i + 1 < TOT:
                    ot_tiles[i + 1] = opool.tile(
                        [2 * OBS, COLS // 2], TRUNK_DT, tag="ot",
                        name="ot%d" % (i + 1),
                    )
                    nc.sync.dma_start(ot_tiles[i + 1][:],
                                      ot_d[:, (i + 1) % NST, :])
                if i < TOT:
                    if i % NST == 0:
                        out_sbs[i // NST] = obpool.tile(
                            [D, NST * GROUPS * C], F32, tag="osb",
                            name="osb%d" % (i // NST),
                        )
                    h0s[i] = hpool.tile([D, COLS], TRUNK_DT, tag="h0",
                                        name="h0_%d" % i)
                    emit_enc(i, ot_tiles.pop(i), h0s[i])
                if i >= 3:
                    st = i - 3
                    emit_tail(st % NST, h3s.pop(st), out_sbs[st // NST])
                    if st % NST == NST - 1:
                        # contiguous store (128 x 32KB descriptors) on the
                        # ACT HWDGE queue so it never head-of-line blocks
                        # the next pass's input loads on the SP queue
                        nc.scalar.dma_start(
                            out_d[:], out_sbs.pop(st // NST)[:])
                if 1 <= i <= TOT:
                    h2s[i - 1] = hpool.tile([D, COLS], TRUNK_DT, tag="h2",
                                            name="h2_%d" % (i - 1))
                    emit_fc(2, h1s.pop(i - 1), h2s[i - 1])
                if 2 <= i <= TOT + 1:
                    h3s[i - 2] = hpool.tile([D, COLS], BF16, tag="h3",
                                            name="h3_%d" % (i - 2))
                    emit_fc(3, h2s.pop(i - 2), h3s[i - 2])
                if i < TOT:
                    h1s[i] = hpool.tile([D, COLS], TRUNK_DT, tag="h1",
                                        name="h1_%d" % i)
                    emit_fc(1, h0s.pop(i), h1s[i])

    nc.compile()
    return nc


def _prep_inputs(inputs):
    """Host-side: fused weights + per-core transposed input shards."""
    f64 = lambda x: np.asarray(x, np.float64)
    enc_w, enc_b = f64(inputs["enc_w"]), f64(inputs["enc_b"])
    fc1_w, fc1_b = f64(inputs["fc1_w"]), f64(inputs["fc1_b"])
    fc2_w, fc2_b = f64(inputs["fc2_w"]), f64(inputs["fc2_b"])
    fc3_w, fc3_b = f64(inputs["fc3_w"]), f64(inputs["fc3_b"])
    cl4_w, cl4_b = f64(inputs["cl4_w"]), f64(inputs["cl4_b"])
    dec_w, dec_b = f64(inputs["dec_w"]), f64(inputs["dec_b"])

    A_ = cl4_w[:D]
    Bm = cl4_w[D:]
    Da = dec_w.reshape(A, D, C)
    Dsum = Da.sum(0)
    Wz = np.concatenate(
        [A_ @ Da[a] + 0.25 * (Bm @ (Dsum - Da[a])) for a in range(A)], axis=1
    )  # [128, 64]
    bias_p = dec_b + cl4_b @ Dsum  # [16]

    import ml_dtypes

    # exp(class bias), replicated per partition — folded into softmax
    eb = np.tile(np.exp(bias_p).astype(np.float32), (D, 1))

    common = {
        "enc_w": np.ascontiguousarray(np.vstack([enc_w, enc_w]), np.float32),
        "w1": np.ascontiguousarray(0.5 * fc1_w, np.float32),
        "w2": np.ascontiguousarray(fc2_w, np.float32),
        "w3": np.ascontiguousarray(fc3_w, np.float32),
        "wz": np.ascontiguousarray(Wz).astype(ml_dtypes.bfloat16),
        "eb": eb,
        "b0": (0.5 * enc_b).astype(np.float32).reshape(D, 1),
        "b1": (fc1_b + 0.5 * fc1_w.sum(0)).astype(np.float32).reshape(D, 1),
        "b2": fc2_b.astype(np.float32).reshape(D, 1),
        "b3": fc3_b.astype(np.float32).reshape(D, 1),
    }

    O = np.asarray(inputs["O"], np.float32)  # [B, A, OBS]
    in_maps = []
    for c in range(NCORES):
        oc = O[c * BLOC : (c + 1) * BLOC]                  # [BLOC, A, OBS]
        # ot[h*64+f, st, a*512+s'] = O[st*1024 + h*512 + s', a, f]
        x = oc.reshape(NST, 2, ST // 2, A, OBS)
        ot = np.ascontiguousarray(x.transpose(1, 4, 0, 3, 2)).reshape(
            2 * OBS, NST, COLS // 2
        )
        in_maps.append({"ot": ot, **common})
    return in_maps


def build(repeats=1):
    key = "nc%d" % repeats
    if key not in _compiled:
        _compiled[key] = _build_bass(repeats)
    return _compiled[key]


def kernel(**inputs):
    nc = build(1)
    in_maps = _prep_inputs(inputs)
    res = run_bass_kernel_spmd(nc, in_maps, core_ids=list(range(NCORES)))
    # device layout [p, st*G*C]: sample (st*1024 + g*128 + p), class c sits
    # at [p, st*128 + g*16 + c] — undo on host
    outs = [
        res.results[i]["probs"]
        .reshape(D, NST, GROUPS, C)
        .transpose(1, 2, 0, 3)
        .reshape(BLOC, C)
        for i in range(NCORES)
    ]
    return np.ascontiguousarray(np.concatenate(outs, axis=0))



# revision 28
# speedup vs baseline: 1.5470x; 1.0108x over previous
"""Trainium2 Bass kernel for nn_CommNetActor.

Network (per sample, 4 agents, all weights shared across agents):
    H0 = sigmoid(O @ enc_w + enc_b)            [B,4,128]
    H1..H3 = relu chain of 128x128 fc layers
    C  = (sum_j H3[:,j] - H3) / 4              (CommNet neighbour mean)
    H4 = [H3 | C] @ cl4_w + cl4_b
    logits = H4.reshape(B,512) @ dec_w + dec_b
    out = softmax(logits)                      [B,16]

Key algebraic fold used here: since C is linear in the H3 agent slices,
the whole tail (neighbour mean + cl4 + dec) collapses into per-agent
readout matrices applied directly to H3:
    logits[b] = sum_a H3[b,a] @ Wz_a + bias'
    Wz_a  = cl4_w[:128] @ D_a + 0.25 * cl4_w[128:] @ (sum_j D_j - D_a)
    bias' = dec_b + cl4_b @ sum_j D_j,      D_a = dec_w[128a:128a+128]
This removes ~35% of the FLOPs, the cross-agent reduction, and the
concat entirely.

Sigmoid is rewritten as tanh so every ScalarE function used (tanh,
relu, exp) lives in one activation-table set:
    sigmoid(x) = 0.5 + 0.5 tanh(x/2)
    H0 := tanh(0.5 x + 0.5 enc_b);  fc1 folded: W1' = 0.5 W1,
    b1' = fc1_b + 0.5 colsum(fc1_w)

Layout: pure data parallelism over 8 cores (8192 samples each). All
activations live transposed in SBUF as [feature(=partition), column],
columns agent-planar per 1024-sample super-tile; sample s of agent a
sits at column (s//512)*2048 + a*512 + s%512. The input is
pre-transposed on the host (no on-device transpose), packed two
samples per column ([128, 2048] per super-tile) so the input DMA uses
all 128 partitions and the K=64 enc matmul runs as two concurrent
row-group-tiled matmuls. Trunk matmuls use float32r (full fp32
storage, 1 cycle/row PE path, measured ~1e-4 rel err end-to-end).
The readout runs activation-stationary (lhsT = H3 chunk in bf16 so
fast-weight-load applies), producing logits in natural [sample, class]
orientation, so softmax is a plain free-dim reduction. The class bias
is folded in as exp(bias') on the GPSIMD/Pool engine inside the
softmax (probs = e^l e^b / sum e^l e^b), removing 8 PE matmuls per
super-tile. ScalarE uses only {tanh, relu, exp} = one activation-table
set (sigmoid was rewritten as tanh with the affine folded into fc1's
weights).

Output leaves the device as [feature-partition, st*group*class] so the
store is one DMA of 128 contiguous 32KB per-partition runs; the 64B
per-sample scatter it replaces cost ~20ns/descriptor of real HWDGE
descriptor-generation (8 x 1024 descriptors ~ 160us/pass, the dominant
term of the previous kernel). kernel() reorders rows on the host.
"""

import numpy as np

import concourse.bass as bass
import concourse.mybir as mybir
import concourse.tile as tile
from concourse import bacc
from concourse.bass import ts
from concourse.bass_utils import run_bass_kernel_spmd

# ---- problem constants (hardcoded per the task contract) ----
B = 65536
A = 4
OBS = 64
D = 128
C = 16
NCORES = 8
BLOC = B // NCORES          # samples per core
ST = 1024                   # samples per super-tile
NST = BLOC // ST
COLS = A * ST               # transposed columns per super-tile
NCHUNK = 512                # matmul moving-dim chunk (one f32 PSUM bank)
GROUPS = ST // D            # 128-sample readout chunks per super-tile

F32 = mybir.dt.float32
F32R = mybir.dt.float32r    # full fp32 storage, fast PE path
BF16 = mybir.dt.bfloat16
AFT = mybir.ActivationFunctionType
ALU = mybir.AluOpType

# matmul input dtype for the main trunk: F32R (fast, ~fp32 storage) or
# F32 (4x slower PE, bit-accurate) or BF16.
TRUNK_DT = F32R

_compiled = {}


# fc-layer block schedules: "A2" = 1024-col block consumed by ACT,
# "D" = 512-col chunk consumed by DVE. Keyed by the number of ACT
# fc-blocks per super-tile (plus the fixed 4 enc blocks + exp on ACT).
SCHEDS = {
    5: {
        "fc1": ("D",) * 8,
        "fc2": ("A2", "D", "D", "A2", "D", "D"),
        "fc3": ("A2", "D", "D", "A2", "A2"),
    },
    4: {
        "fc1": ("D",) * 8,
        "fc2": ("A2", "D", "D", "D", "D", "A2"),
        "fc3": ("A2", "D", "D", "D", "D", "A2"),
    },
    6: {
        "fc1": ("A2", "D", "D", "D", "D", "D", "D"),
        "fc2": ("A2", "D", "D", "A2", "D", "D"),
        "fc3": ("A2", "A2", "D", "D", "A2"),
    },
}


def _build_bass(repeats=1, nact=5):
    # Bacc (not plain Bass): its compile() runs generate_event_semaphores /
    # move_matmul_waits_to_ldweights, which legalize multi-wait instructions
    # down to the TRN2 limit of one sync wait per instruction.
    # repeats>1 re-emits the whole pipeline (incl. const loads) that many
    # times in one NEFF — used by test.py to measure marginal per-pass
    # device time without the ~85ms axon dispatch round-trip.
    nc = bacc.Bacc()

    # Input packed two-samples-per-column: partitions 0-63 hold features of
    # the first half of each super-tile's samples, 64-127 the second half.
    # Full 128-partition DMA + the enc matmul runs as two concurrent
    # row-group-tiled K=64 matmuls (tile_position (0,0) / (64,0)).
    ot_d = nc.dram_tensor("ot", [2 * OBS, NST, COLS // 2], TRUNK_DT, kind="ExternalInput")
    ew_d = nc.dram_tensor("enc_w", [2 * OBS, D], TRUNK_DT, kind="ExternalInput")
    w1_d = nc.dram_tensor("w1", [D, D], TRUNK_DT, kind="ExternalInput")
    w2_d = nc.dram_tensor("w2", [D, D], TRUNK_DT, kind="ExternalInput")
    w3_d = nc.dram_tensor("w3", [D, D], TRUNK_DT, kind="ExternalInput")
    wz_d = nc.dram_tensor("wz", [D, A * C], BF16, kind="ExternalInput")
    # class bias folded into softmax as exp(bias): probs = e^l e^b / sum
    # (replicated per partition). Pool is nearly idle, so the extra multiply
    # there is free and PE drops its 8 rank-1 bias matmuls per super-tile.
    eb_d = nc.dram_tensor("eb", [D, C], F32, kind="ExternalInput")
    b0_d = nc.dram_tensor("b0", [D, 1], F32, kind="ExternalInput")
    b1_d = nc.dram_tensor("b1", [D, 1], F32, kind="ExternalInput")
    b2_d = nc.dram_tensor("b2", [D, 1], F32, kind="ExternalInput")
    b3_d = nc.dram_tensor("b3", [D, 1], F32, kind="ExternalInput")
    # Output layout [feature-partition, st, group, class] so the store is one
    # DMA of 128 contiguous 32KB per-partition runs (128 descriptors) instead
    # of 8 scatters of 1024 64B descriptors (HW DGE descriptor-gen dominated
    # the measured device time). Host reorders to [BLOC, C] afterwards.
    out_d = nc.dram_tensor("probs", [D, NST * GROUPS * C], F32, kind="ExternalOutput")

    with tile.TileContext(nc) as tc:
        with (
            tc.tile_pool(name="consts", bufs=1) as cpool,
            tc.tile_pool(name="ot", bufs=2) as opool,
            tc.tile_pool(name="acts", bufs=2) as hpool,
            tc.tile_pool(name="soft", bufs=2) as spool,
            tc.tile_pool(name="osb", bufs=2) as obpool,
            tc.tile_pool(name="mmA", bufs=2, space="PSUM") as mmpoolA,
            tc.tile_pool(name="mmD", bufs=3, space="PSUM") as mmpoolD,
            tc.tile_pool(name="lg", bufs=1, space="PSUM") as lgpool,
        ):
            # ot(0) goes FIRST on the SP DMA queue: every const behind it
            # would otherwise delay the first enc matmul by ~1us of DGE
            # dispatch each (measured 12us fill stall). enc deps (ew, b0)
            # follow immediately; deeper-layer weights land later, which is
            # fine because their consumers start later anyway.
            ot0_t = opool.tile([2 * OBS, COLS // 2], TRUNK_DT, tag="ot",
                               name="ot0")
            nc.sync.dma_start(ot0_t[:], ot_d[:, 0, :])
            ew_t = cpool.tile([2 * OBS, D], TRUNK_DT, name="ew")
            nc.sync.dma_start(ew_t[:], ew_d[:])
            b_t = {}
            for nm, dd in (("b0", b0_d), ("b1", b1_d), ("b2", b2_d), ("b3", b3_d)):
                b_t[nm] = cpool.tile([D, 1], F32, name=nm)
                nc.sync.dma_start(b_t[nm][:], dd[:])
            w_t = {}
            for nm, dd in (("w1", w1_d), ("w2", w2_d), ("w3", w3_d)):
                w_t[nm] = cpool.tile([D, D], TRUNK_DT, name=nm)
                nc.sync.dma_start(w_t[nm][:], dd[:])
            wz_t = cpool.tile([D, A * C], BF16, name="wz")
            nc.sync.dma_start(wz_t[:], wz_d[:])
            eb_t = cpool.tile([D, C], F32, name="eb")
            nc.sync.dma_start(eb_t[:], eb_d[:])

            # PSUM can only be read by ACT and DVE (GPSIMD/Pool and the DMA
            # engines are locked out by the BIR verifier — verified: walrus
            # rejects a Pool PSUM read), so all post-matmul relu flows
            # through those two. ACT takes 1024-col blocks (its PSUM-access
            # overhead amortizes), DVE 512-col chunks. The pools are split
            # BY CONSUMER ENGINE: a shared pool lets a run of same-engine
            # blocks head-of-line block the in-order PE (measured +20us in
            # sim). Per ST: ACT = 4 enc + 5 fc blocks + exp ~= 9.6us, DVE =
            # 14 fc chunks + softmax reduce/reciprocal ~= 9.9us. Pool
            # (GPSIMD, SBUF-only) takes the two softmax multiplies.
            # "A2" entries are 1024-col blocks (2 matmul chunks), "D"
            # entries 512-col chunks.
            SCHED = SCHEDS[nact]

            def emit_tail(st, h3, out_sb):
                """Readout + softmax for a finished super-tile.

                Emitted at the TOP of the next iteration (software pipeline):
                its 40 tiny readout matmuls depend only on old data, so the
                in-order PE queue drains them instantly instead of stalling
                the next super-tile's enc matmuls behind the slow fc3 tail.
                Probs land in out_sb (one persistent SBUF tile); a single
                contiguous DMA stores everything after the last super-tile.
                """
                lg = lgpool.tile([D, GROUPS * C], F32, tag="lg")
                for g in range(GROUPS):
                    cbase = (g // 4) * 2048 + (g % 4) * D
                    for a in range(A):
                        nc.tensor.matmul(
                            lg[:, ts(g, C)],
                            h3[:, cbase + a * 512 : cbase + a * 512 + D],
                            wz_t[:, ts(a, C)],
                            start=(a == 0), stop=(a == A - 1),
                        )
                e = spool.tile([D, GROUPS * C], F32, tag="e")
                nc.scalar.activation(e[:], lg[:], AFT.Exp)
                # fold the class bias in as exp(b) on the idle Pool engine
                e2 = spool.tile([D, GROUPS * C], F32, tag="e2")
                nc.gpsimd.tensor_mul(
                    e2[:].rearrange("p (g c) -> p g c", c=C),
                    e[:].rearrange("p (g c) -> p g c", c=C),
                    eb_t[:].unsqueeze(1).broadcast_to([D, GROUPS, C]),
                )
                s = spool.tile([D, GROUPS], F32, tag="s")
                nc.vector.reduce_sum(
                    s[:], e2[:].rearrange("p (g c) -> p g c", c=C),
                    axis=mybir.AxisListType.X,
                )
                r = spool.tile([D, GROUPS], F32, tag="r")
                nc.vector.reciprocal(r[:], s[:])
                nc.gpsimd.tensor_mul(
                    out_sb[:, st * GROUPS * C : (st + 1) * GROUPS * C]
                    .rearrange("p (g c) -> p g c", c=C),
                    e2[:].rearrange("p (g c) -> p g c", c=C),
                    r[:].unsqueeze(2).broadcast_to([D, GROUPS, C]),
                )

            def emit_enc(st, ot_t, h0):
                # enc: tanh(0.5 x + 0.5 b). block j: partition-half hh=j//2
                # of the packed input, 1024 input cols at (j%2)*1024 ->
                # h0 cols j*1024. ACT processes 1024-col blocks.
                for j in range(4):
                    hh = j // 2
                    base = (j % 2) * 1024
                    ps = mmpoolA.tile([D, 1024], F32, tag="mm")
                    for k in range(2):
                        nc.tensor.matmul(
                            ps[:, ts(k, NCHUNK)],
                            ew_t[64 * hh : 64 * (hh + 1), :],
                            ot_t[64 * hh : 64 * (hh + 1),
                                 base + k * NCHUNK : base + (k + 1) * NCHUNK],
                            start=True, stop=True,
                        )
                    nc.scalar.activation(
                        h0[:, ts(j, 1024)], ps[:], AFT.Tanh,
                        bias=b_t["b0"][:], scale=0.5,
                    )

            def emit_fc(layer, src, dst):
                wname = "w%d" % layer
                bname = "b%d" % layer
                col = 0
                for ent in SCHED["fc%d" % layer]:
                    if ent == "A2":
                        ps = mmpoolA.tile([D, 1024], F32, tag="mm")
                        for k in range(2):
                            nc.tensor.matmul(
                                ps[:, ts(k, NCHUNK)],
                                w_t[wname][:],
                                src[:, col + k * NCHUNK : col + (k + 1) * NCHUNK],
                                start=True, stop=True,
                            )
                        nc.scalar.activation(
                            dst[:, col : col + 1024], ps[:], AFT.Relu,
                            bias=b_t[bname][:],
                        )
                        col += 1024
                    else:
                        ps = mmpoolD.tile([D, NCHUNK], F32, tag="mm")
                        nc.tensor.matmul(
                            ps[:],
                            w_t[wname][:],
                            src[:, col : col + NCHUNK],
                            start=True, stop=True,
                        )
                        nc.vector.tensor_scalar(
                            dst[:, col : col + NCHUNK], ps[:],
                            b_t[bname][:], 0.0, ALU.add, ALU.max,
                        )
                        col += NCHUNK
                assert col == COLS

            # ---- 4-deep skewed software pipeline over super-tiles ----
            # Iteration i emits: enc(i), tail(i-3), fc2(i-1), fc3(i-2),
            # fc1(i). Every engine is in-order, so ops must be emitted in
            # the order they become READY: layer L of super-tile st is only
            # emitted one iteration after layer L-1 of st, which means the
            # ACT/DVE/Pool chunks of st's deep layers never block the next
            # super-tile's enc/fc1 stream (the baseline's serialization).
            # ---- continuous 4-deep skewed software pipeline over all
            # repeats * NST super-tiles.  With repeats=1 this is a plain
            # single pass; with repeats>1 consecutive passes overlap exactly
            # like back-to-back kernel invocations with resident weights,
            # which is what the marginal-repeat timing measures. ----
            TOT = repeats * NST
            ot_tiles, h0s, h1s, h2s, h3s = {}, {}, {}, {}, {}
            ot_tiles[0] = ot0_t
            out_sbs = {}

            for i in range(TOT + 3):
                # prefetch first so nothing delays the next input tile
                if i + 1 < TOT:
                    ot_tiles[i + 1] = opool.tile(
                        [2 * OBS, COLS // 2], TRUNK_DT, tag="ot",
                        name="ot%d" % (i + 1),
                    )
                    nc.sync.dma_start(ot_tiles[i + 1][:],
                                      ot_d[:, (i + 1) % NST, :])
                if i < TOT:
                    if i % NST == 0:
                        out_sbs[i // NST] = obpool.tile(
                            [D, NST * GROUPS * C], F32, tag="osb",
                            name="osb%d" % (i // NST),
                        )
                    h0s[i] = hpool.tile([D, COLS], TRUNK_DT, tag="h0",
                                        name="h0_%d" % i)
                    emit_enc(i, ot_tiles.pop(i), h0s[i])
                if i >= 3:
                    st = i - 3
                    emit_tail(st % NST, h3s.pop(st), out_sbs[st // NST])
                    if st % NST == NST - 1:
                        # contiguous store (128 x 32KB descriptors) on the
                        # ACT HWDGE queue so it never head-of-line blocks
                        # the next pass's input loads on the SP queue
                        nc.scalar.dma_start(
                            out_d[:], out_sbs.pop(st // NST)[:])
                if 1 <= i <= TOT:
                    h2s[i - 1] = hpool.tile([D, COLS], TRUNK_DT, tag="h2",
                                            name="h2_%d" % (i - 1))
                    emit_fc(2, h1s.pop(i - 1), h2s[i - 1])
                if 2 <= i <= TOT + 1:
                    h3s[i - 2] = hpool.tile([D, COLS], BF16, tag="h3",
                                            name="h3_%d" % (i - 2))
                    emit_fc(3, h2s.pop(i - 2), h3s[i - 2])
                if i < TOT:
                    h1s[i] = hpool.tile([D, COLS], TRUNK_DT, tag="h1",
                                        name="h1_%d" % i)
                    emit_fc(1, h0s.pop(i), h1s[i])

    nc.compile()
    return nc


def _prep_inputs(inputs):
    """Host-side: fused weights + per-core transposed input shards."""
    f64 = lambda x: np.asarray(x, np.float64)
    enc_w, enc_b = f64(inputs["enc_w"]), f64(inputs["enc_b"])
    fc1_w, fc1_b = f64(inputs["fc1_w"]), f64(inputs["fc1_b"])
    fc2_w, fc2_b = f64(inputs["fc2_w"]), f64(inputs["fc2_b"])
    fc3_w, fc3_b = f64(inputs["fc3_w"]), f64(inputs["fc3_b"])
    cl4_w, cl4_b = f64(inputs["cl4_w"]), f64(inputs["cl4_b"])
    dec_w, dec_b = f64(inputs["dec_w"]), f64(inputs["dec_b"])

    A_ = cl4_w[:D]
    Bm = cl4_w[D:]
    Da = dec_w.reshape(A, D, C)
    Dsum = Da.sum(0)
    Wz = np.concatenate(
        [A_ @ Da[a] + 0.25 * (Bm @ (Dsum - Da[a])) for a in range(A)], axis=1
    )  # [128, 64]
    bias_p = dec_b + cl4_b @ Dsum  # [16]

    import ml_dtypes

    # exp(class bias), replicated per partition — folded into softmax
    eb = np.tile(np.exp(bias_p).astype(np.float32), (D, 1))

    common = {
        "enc_w": np.ascontiguousarray(np.vstack([enc_w, enc_w]), np.float32),
        "w1": np.ascontiguousarray(0.5 * fc1_w, np.float32),
        "w2": np.ascontiguousarray(fc2_w, np.float32),
        "w3": np.ascontiguousarray(fc3_w, np.float32),
        "wz": np.ascontiguousarray(Wz).astype(ml_dtypes.bfloat16),
        "eb": eb,
        "b0": (0.5 * enc_b).astype(np.float32).reshape(D, 1),
        "b1": (fc1_b + 0.5 * fc1_w.sum(0)).astype(np.float32).reshape(D, 1),
        "b2": fc2_b.astype(np.float32).reshape(D, 1),
        "b3": fc3_b.astype(np.float32).reshape(D, 1),
    }

    O = np.asarray(inputs["O"], np.float32)  # [B, A, OBS]
    in_maps = []
    for c in range(NCORES):
        oc = O[c * BLOC : (c + 1) * BLOC]                  # [BLOC, A, OBS]
        # ot[h*64+f, st, a*512+s'] = O[st*1024 + h*512 + s', a, f]
        x = oc.reshape(NST, 2, ST // 2, A, OBS)
        ot = np.ascontiguousarray(x.transpose(1, 4, 0, 3, 2)).reshape(
            2 * OBS, NST, COLS // 2
        )
        in_maps.append({"ot": ot, **common})
    return in_maps


def build(repeats=1, nact=5):
    key = "nc%d_%d" % (repeats, nact)
    if key not in _compiled:
        _compiled[key] = _build_bass(repeats, nact)
    return _compiled[key]


def kernel(**inputs):
    nc = build(1)
    in_maps = _prep_inputs(inputs)
    res = run_bass_kernel_spmd(nc, in_maps, core_ids=list(range(NCORES)))
    # device layout [p, st*G*C]: sample (st*1024 + g*128 + p), class c sits
    # at [p, st*128 + g*16 + c] — undo on host
    outs = [
        res.results[i]["probs"]
        .reshape(D, NST, GROUPS, C)
        .transpose(1, 2, 0, 3)
        .reshape(BLOC, C)
        for i in range(NCORES)
    ]
    return np.ascontiguousarray(np.concatenate(outs, axis=0))



# revision 36
# speedup vs baseline: 1.6089x; 1.0400x over previous
"""Trainium2 Bass kernel for nn_CommNetActor.

Network (per sample, 4 agents, all weights shared across agents):
    H0 = sigmoid(O @ enc_w + enc_b)            [B,4,128]
    H1..H3 = relu chain of 128x128 fc layers
    C  = (sum_j H3[:,j] - H3) / 4              (CommNet neighbour mean)
    H4 = [H3 | C] @ cl4_w + cl4_b
    logits = H4.reshape(B,512) @ dec_w + dec_b
    out = softmax(logits)                      [B,16]

Key algebraic fold used here: since C is linear in the H3 agent slices,
the whole tail (neighbour mean + cl4 + dec) collapses into per-agent
readout matrices applied directly to H3:
    logits[b] = sum_a H3[b,a] @ Wz_a + bias'
    Wz_a  = cl4_w[:128] @ D_a + 0.25 * cl4_w[128:] @ (sum_j D_j - D_a)
    bias' = dec_b + cl4_b @ sum_j D_j,      D_a = dec_w[128a:128a+128]
This removes ~35% of the FLOPs, the cross-agent reduction, and the
concat entirely.

Sigmoid is rewritten as tanh so every ScalarE function used (tanh,
relu, exp) lives in one activation-table set:
    sigmoid(x) = 0.5 + 0.5 tanh(x/2)
    H0 := tanh(0.5 x + 0.5 enc_b);  fc1 folded: W1' = 0.5 W1,
    b1' = fc1_b + 0.5 colsum(fc1_w)

Layout: pure data parallelism over 8 cores (8192 samples each). All
activations live transposed in SBUF as [feature(=partition), column],
columns agent-planar per 1024-sample super-tile; sample s of agent a
sits at column (s//512)*2048 + a*512 + s%512. The input is
pre-transposed on the host (no on-device transpose), packed two
samples per column ([128, 2048] per super-tile) so the input DMA uses
all 128 partitions and the K=64 enc matmul runs as two concurrent
row-group-tiled matmuls. Trunk matmuls use float32r (full fp32
storage, 1 cycle/row PE path, measured ~1e-4 rel err end-to-end).
The readout runs activation-stationary (lhsT = H3 chunk in bf16 so
fast-weight-load applies), producing logits in natural [sample, class]
orientation, so softmax is a plain free-dim reduction. The class bias
is folded in as exp(bias') on the GPSIMD/Pool engine inside the
softmax (probs = e^l e^b / sum e^l e^b), removing 8 PE matmuls per
super-tile. ScalarE uses only {tanh, relu, exp} = one activation-table
set (sigmoid was rewritten as tanh with the affine folded into fc1's
weights).

Output leaves the device as [feature-partition, st*group*class] so the
store is one DMA of 128 contiguous 32KB per-partition runs; the 64B
per-sample scatter it replaces cost ~20ns/descriptor of real HWDGE
descriptor-generation (8 x 1024 descriptors ~ 160us/pass, the dominant
term of the previous kernel). kernel() reorders rows on the host.
"""

import numpy as np

import concourse.bass as bass
import concourse.mybir as mybir
import concourse.tile as tile
from concourse import bacc
from concourse.bass import ts
from concourse.bass_utils import run_bass_kernel_spmd

# ---- problem constants (hardcoded per the task contract) ----
B = 65536
A = 4
OBS = 64
D = 128
C = 16
NCORES = 8
BLOC = B // NCORES          # samples per core
ST = 1024                   # samples per super-tile
NST = BLOC // ST
COLS = A * ST               # transposed columns per super-tile
NCHUNK = 512                # matmul moving-dim chunk (one f32 PSUM bank)
GROUPS = ST // D            # 128-sample readout chunks per super-tile

F32 = mybir.dt.float32
F32R = mybir.dt.float32r    # full fp32 storage, fast PE path
BF16 = mybir.dt.bfloat16
AFT = mybir.ActivationFunctionType
ALU = mybir.AluOpType

# matmul input dtype for the main trunk: F32R (fast, ~fp32 storage) or
# F32 (4x slower PE, bit-accurate) or BF16.
TRUNK_DT = F32R

_compiled = {}


# fc-layer block schedules: "A2" = 1024-col block consumed by ACT,
# "D" = 512-col chunk consumed by DVE. Keyed by the number of ACT
# fc-blocks per super-tile (plus the fixed 4 enc blocks + exp on ACT).
SCHEDS = {
    5: {
        "fc1": ("D",) * 8,
        "fc2": ("A2", "D", "D", "A2", "D", "D"),
        "fc3": ("A2", "D", "D", "A2", "A2"),
    },
    # 1024-col DVE chunks: 7 DVE instructions/ST instead of 14 (PSUM-access
    # overhead amortizes over 1024 cols). Needs mmD [D,1024] tiles, which
    # only fit if lg folds into the mmA pool. HW A/B: 75.6us vs 76.2us for
    # nact=5 — statistical tie (the DVE saving is offset by lg/mmA pool
    # contention), so nact=5 stays the default.
    50: {
        "fc1": ("D2", "D2", "D2", "D2"),
        "fc2": ("A2", "D2", "D2", "A2"),
        "fc3": ("A2", "D2", "A2", "A2"),
    },
    4: {
        "fc1": ("D",) * 8,
        "fc2": ("A2", "D", "D", "D", "D", "A2"),
        "fc3": ("A2", "D", "D", "D", "D", "A2"),
    },
    6: {
        "fc1": ("A2", "D", "D", "D", "D", "D", "D"),
        "fc2": ("A2", "D", "D", "A2", "D", "D"),
        "fc3": ("A2", "A2", "D", "D", "A2"),
    },
}


def _build_bass(repeats=1, nact=5):
    # Bacc (not plain Bass): its compile() runs generate_event_semaphores /
    # move_matmul_waits_to_ldweights, which legalize multi-wait instructions
    # down to the TRN2 limit of one sync wait per instruction.
    # repeats>1 re-emits the whole pipeline (incl. const loads) that many
    # times in one NEFF — used by test.py to measure marginal per-pass
    # device time without the ~85ms axon dispatch round-trip.
    nc = bacc.Bacc()

    # Input packed two-samples-per-column: partitions 0-63 hold features of
    # the first half of each super-tile's samples, 64-127 the second half.
    # Full 128-partition DMA + the enc matmul runs as two concurrent
    # row-group-tiled K=64 matmuls (tile_position (0,0) / (64,0)).
    ot_d = nc.dram_tensor("ot", [2 * OBS, NST, COLS // 2], TRUNK_DT, kind="ExternalInput")
    ew_d = nc.dram_tensor("enc_w", [2 * OBS, D], TRUNK_DT, kind="ExternalInput")
    w1_d = nc.dram_tensor("w1", [D, D], TRUNK_DT, kind="ExternalInput")
    w2_d = nc.dram_tensor("w2", [D, D], TRUNK_DT, kind="ExternalInput")
    w3_d = nc.dram_tensor("w3", [D, D], TRUNK_DT, kind="ExternalInput")
    wz_d = nc.dram_tensor("wz", [D, A * C], BF16, kind="ExternalInput")
    # class bias folded into softmax as exp(bias): probs = e^l e^b / sum
    # (replicated per partition). Pool is nearly idle, so the extra multiply
    # there is free and PE drops its 8 rank-1 bias matmuls per super-tile.
    eb_d = nc.dram_tensor("eb", [D, C], F32, kind="ExternalInput")
    b0_d = nc.dram_tensor("b0", [D, 1], F32, kind="ExternalInput")
    b1_d = nc.dram_tensor("b1", [D, 1], F32, kind="ExternalInput")
    b2_d = nc.dram_tensor("b2", [D, 1], F32, kind="ExternalInput")
    b3_d = nc.dram_tensor("b3", [D, 1], F32, kind="ExternalInput")
    # Output layout [feature-partition, st, group, class] so the store is one
    # DMA of 128 contiguous 32KB per-partition runs (128 descriptors) instead
    # of 8 scatters of 1024 64B descriptors (HW DGE descriptor-gen dominated
    # the measured device time). Host reorders to [BLOC, C] afterwards.
    out_d = nc.dram_tensor("probs", [D, NST * GROUPS * C], F32, kind="ExternalOutput")

    with tile.TileContext(nc) as tc:
        with (
            tc.tile_pool(name="consts", bufs=1) as cpool,
            tc.tile_pool(name="ot", bufs=2) as opool,
            tc.tile_pool(name="acts", bufs=2) as hpool,
            tc.tile_pool(name="soft", bufs=2) as spool,
            tc.tile_pool(name="osb", bufs=2) as obpool,
            tc.tile_pool(name="mmA", bufs=2, space="PSUM") as mmpoolA,
            tc.tile_pool(name="mmD", bufs=3 if nact != 50 else 2,
                         space="PSUM") as mmpoolD,
            tc.tile_pool(name="lg", bufs=1, space="PSUM") as lgpool,
        ):
            # ot(0) goes FIRST on the SP DMA queue: every const behind it
            # would otherwise delay the first enc matmul by ~1us of DGE
            # dispatch each (measured 12us fill stall). enc deps (ew, b0)
            # follow immediately; deeper-layer weights land later, which is
            # fine because their consumers start later anyway.
            ot0_t = opool.tile([2 * OBS, COLS // 2], TRUNK_DT, tag="ot",
                               name="ot0")
            nc.sync.dma_start(ot0_t[:], ot_d[:, 0, :])
            ew_t = cpool.tile([2 * OBS, D], TRUNK_DT, name="ew")
            nc.sync.dma_start(ew_t[:], ew_d[:])
            b_t = {}
            for nm, dd in (("b0", b0_d), ("b1", b1_d), ("b2", b2_d), ("b3", b3_d)):
                b_t[nm] = cpool.tile([D, 1], F32, name=nm)
                nc.sync.dma_start(b_t[nm][:], dd[:])
            w_t = {}
            for nm, dd in (("w1", w1_d), ("w2", w2_d), ("w3", w3_d)):
                w_t[nm] = cpool.tile([D, D], TRUNK_DT, name=nm)
                nc.sync.dma_start(w_t[nm][:], dd[:])
            wz_t = cpool.tile([D, A * C], BF16, name="wz")
            nc.sync.dma_start(wz_t[:], wz_d[:])
            eb_t = cpool.tile([D, C], F32, name="eb")
            nc.sync.dma_start(eb_t[:], eb_d[:])

            # PSUM can only be read by ACT and DVE (GPSIMD/Pool and the DMA
            # engines are locked out by the BIR verifier — verified: walrus
            # rejects a Pool PSUM read), so all post-matmul relu flows
            # through those two. ACT takes 1024-col blocks (its PSUM-access
            # overhead amortizes), DVE 512-col chunks. The pools are split
            # BY CONSUMER ENGINE: a shared pool lets a run of same-engine
            # blocks head-of-line block the in-order PE (measured +20us in
            # sim). Per ST: ACT = 4 enc + 5 fc blocks + exp ~= 9.6us, DVE =
            # 14 fc chunks + softmax reduce/reciprocal ~= 9.9us. Pool
            # (GPSIMD, SBUF-only) takes the two softmax multiplies.
            # "A2" entries are 1024-col blocks (2 matmul chunks), "D"
            # entries 512-col chunks.
            SCHED = SCHEDS[nact]

            def emit_tail(st, h3, out_sb):
                """Readout + softmax for a finished super-tile.

                Emitted at the TOP of the next iteration (software pipeline):
                its 40 tiny readout matmuls depend only on old data, so the
                in-order PE queue drains them instantly instead of stalling
                the next super-tile's enc matmuls behind the slow fc3 tail.
                Probs land in out_sb (one persistent SBUF tile); a single
                contiguous DMA stores everything after the last super-tile.
                """
                # for the 1024-col-DVE schedule the logits tile borrows an
                # mmA rotation slot (frees its PSUM bank for wider mmD tiles);
                # exp (ACT) is its consumer either way, same pool class
                if nact == 50:
                    lg = mmpoolA.tile([D, 1024], F32, tag="mm")
                else:
                    lg = lgpool.tile([D, GROUPS * C], F32, tag="lg")
                for g in range(GROUPS):
                    cbase = (g // 4) * 2048 + (g % 4) * D
                    for a in range(A):
                        nc.tensor.matmul(
                            lg[:, ts(g, C)],
                            h3[:, cbase + a * 512 : cbase + a * 512 + D],
                            wz_t[:, ts(a, C)],
                            start=(a == 0), stop=(a == A - 1),
                        )
                e = spool.tile([D, GROUPS * C], F32, tag="e")
                nc.scalar.activation(e[:], lg[:, 0 : GROUPS * C], AFT.Exp)
                # fold the class bias in as exp(b) on the idle Pool engine
                e2 = spool.tile([D, GROUPS * C], F32, tag="e2")
                nc.gpsimd.tensor_mul(
                    e2[:].rearrange("p (g c) -> p g c", c=C),
                    e[:].rearrange("p (g c) -> p g c", c=C),
                    eb_t[:].unsqueeze(1).broadcast_to([D, GROUPS, C]),
                )
                s = spool.tile([D, GROUPS], F32, tag="s")
                nc.vector.reduce_sum(
                    s[:], e2[:].rearrange("p (g c) -> p g c", c=C),
                    axis=mybir.AxisListType.X,
                )
                r = spool.tile([D, GROUPS], F32, tag="r")
                nc.vector.reciprocal(r[:], s[:])
                nc.gpsimd.tensor_mul(
                    out_sb[:, st * GROUPS * C : (st + 1) * GROUPS * C]
                    .rearrange("p (g c) -> p g c", c=C),
                    e2[:].rearrange("p (g c) -> p g c", c=C),
                    r[:].unsqueeze(2).broadcast_to([D, GROUPS, C]),
                )

            def emit_enc(st, ot_t, h0):
                # enc: tanh(0.5 x + 0.5 b). block j: partition-half hh=j//2
                # of the packed input, 1024 input cols at (j%2)*1024 ->
                # h0 cols j*1024. ACT processes 1024-col blocks.
                for j in range(4):
                    hh = j // 2
                    base = (j % 2) * 1024
                    ps = mmpoolA.tile([D, 1024], F32, tag="mm")
                    for k in range(2):
                        nc.tensor.matmul(
                            ps[:, ts(k, NCHUNK)],
                            ew_t[64 * hh : 64 * (hh + 1), :],
                            ot_t[64 * hh : 64 * (hh + 1),
                                 base + k * NCHUNK : base + (k + 1) * NCHUNK],
                            start=True, stop=True,
                        )
                    nc.scalar.activation(
                        h0[:, ts(j, 1024)], ps[:], AFT.Tanh,
                        bias=b_t["b0"][:], scale=0.5,
                    )

            def emit_fc(layer, src, dst):
                wname = "w%d" % layer
                bname = "b%d" % layer
                col = 0
                for ent in SCHED["fc%d" % layer]:
                    if ent == "A2":
                        ps = mmpoolA.tile([D, 1024], F32, tag="mm")
                        for k in range(2):
                            nc.tensor.matmul(
                                ps[:, ts(k, NCHUNK)],
                                w_t[wname][:],
                                src[:, col + k * NCHUNK : col + (k + 1) * NCHUNK],
                                start=True, stop=True,
                            )
                        nc.scalar.activation(
                            dst[:, col : col + 1024], ps[:], AFT.Relu,
                            bias=b_t[bname][:],
                        )
                        col += 1024
                    elif ent == "D2":
                        ps = mmpoolD.tile([D, 1024], F32, tag="mm")
                        for k in range(2):
                            nc.tensor.matmul(
                                ps[:, ts(k, NCHUNK)],
                                w_t[wname][:],
                                src[:, col + k * NCHUNK : col + (k + 1) * NCHUNK],
                                start=True, stop=True,
                            )
                        nc.vector.tensor_scalar(
                            dst[:, col : col + 1024], ps[:],
                            b_t[bname][:], 0.0, ALU.add, ALU.max,
                        )
                        col += 1024
                    else:
                        ps = mmpoolD.tile([D, NCHUNK], F32, tag="mm")
                        nc.tensor.matmul(
                            ps[:],
                            w_t[wname][:],
                            src[:, col : col + NCHUNK],
                            start=True, stop=True,
                        )
                        nc.vector.tensor_scalar(
                            dst[:, col : col + NCHUNK], ps[:],
                            b_t[bname][:], 0.0, ALU.add, ALU.max,
                        )
                        col += NCHUNK
                assert col == COLS

            # ---- 4-deep skewed software pipeline over super-tiles ----
            # Iteration i emits: enc(i), tail(i-3), fc2(i-1), fc3(i-2),
            # fc1(i). Every engine is in-order, so ops must be emitted in
            # the order they become READY: layer L of super-tile st is only
            # emitted one iteration after layer L-1 of st, which means the
            # ACT/DVE/Pool chunks of st's deep layers never block the next
            # super-tile's enc/fc1 stream (the baseline's serialization).
            # ---- continuous 4-deep skewed software pipeline over all
            # repeats * NST super-tiles.  With repeats=1 this is a plain
            # single pass; with repeats>1 consecutive passes overlap exactly
            # like back-to-back kernel invocations with resident weights,
            # which is what the marginal-repeat timing measures. ----
            TOT = repeats * NST
            ot_tiles, h0s, h1s, h2s, h3s = {}, {}, {}, {}, {}
            ot_tiles[0] = ot0_t
            out_sbs = {}

            for i in range(TOT + 3):
                # prefetch first so nothing delays the next input tile
                if i + 1 < TOT:
                    ot_tiles[i + 1] = opool.tile(
                        [2 * OBS, COLS // 2], TRUNK_DT, tag="ot",
                        name="ot%d" % (i + 1),
                    )
                    nc.sync.dma_start(ot_tiles[i + 1][:],
                                      ot_d[:, (i + 1) % NST, :])
                if i < TOT:
                    if i % NST == 0:
                        out_sbs[i // NST] = obpool.tile(
                            [D, NST * GROUPS * C], F32, tag="osb",
                            name="osb%d" % (i // NST),
                        )
                    h0s[i] = hpool.tile([D, COLS], TRUNK_DT, tag="h0",
                                        name="h0_%d" % i)
                    emit_enc(i, ot_tiles.pop(i), h0s[i])
                if i >= 3:
                    st = i - 3
                    emit_tail(st % NST, h3s.pop(st), out_sbs[st // NST])
                    if st % NST == NST - 1:
                        # contiguous store (128 x 32KB descriptors) on the
                        # ACT HWDGE queue so it never head-of-line blocks
                        # the next pass's input loads on the SP queue
                        nc.scalar.dma_start(
                            out_d[:], out_sbs.pop(st // NST)[:])
                if 1 <= i <= TOT:
                    h2s[i - 1] = hpool.tile([D, COLS], TRUNK_DT, tag="h2",
                                            name="h2_%d" % (i - 1))
                    emit_fc(2, h1s.pop(i - 1), h2s[i - 1])
                if 2 <= i <= TOT + 1:
                    h3s[i - 2] = hpool.tile([D, COLS], BF16, tag="h3",
                                            name="h3_%d" % (i - 2))
                    emit_fc(3, h2s.pop(i - 2), h3s[i - 2])
                if i < TOT:
                    h1s[i] = hpool.tile([D, COLS], TRUNK_DT, tag="h1",
                                        name="h1_%d" % i)
                    emit_fc(1, h0s.pop(i), h1s[i])

    nc.compile()
    return nc


def _prep_inputs(inputs):
    """Host-side: fused weights + per-core transposed input shards."""
    f64 = lambda x: np.asarray(x, np.float64)
    enc_w, enc_b = f64(inputs["enc_w"]), f64(inputs["enc_b"])
    fc1_w, fc1_b = f64(inputs["fc1_w"]), f64(inputs["fc1_b"])
    fc2_w, fc2_b = f64(inputs["fc2_w"]), f64(inputs["fc2_b"])
    fc3_w, fc3_b = f64(inputs["fc3_w"]), f64(inputs["fc3_b"])
    cl4_w, cl4_b = f64(inputs["cl4_w"]), f64(inputs["cl4_b"])
    dec_w, dec_b = f64(inputs["dec_w"]), f64(inputs["dec_b"])

    A_ = cl4_w[:D]
    Bm = cl4_w[D:]
    Da = dec_w.reshape(A, D, C)
    Dsum = Da.sum(0)
    Wz = np.concatenate(
        [A_ @ Da[a] + 0.25 * (Bm @ (Dsum - Da[a])) for a in range(A)], axis=1
    )  # [128, 64]
    bias_p = dec_b + cl4_b @ Dsum  # [16]

    import ml_dtypes

    # exp(class bias), replicated per partition — folded into softmax
    eb = np.tile(np.exp(bias_p).astype(np.float32), (D, 1))

    common = {
        "enc_w": np.ascontiguousarray(np.vstack([enc_w, enc_w]), np.float32),
        "w1": np.ascontiguousarray(0.5 * fc1_w, np.float32),
        "w2": np.ascontiguousarray(fc2_w, np.float32),
        "w3": np.ascontiguousarray(fc3_w, np.float32),
        "wz": np.ascontiguousarray(Wz).astype(ml_dtypes.bfloat16),
        "eb": eb,
        "b0": (0.5 * enc_b).astype(np.float32).reshape(D, 1),
        "b1": (fc1_b + 0.5 * fc1_w.sum(0)).astype(np.float32).reshape(D, 1),
        "b2": fc2_b.astype(np.float32).reshape(D, 1),
        "b3": fc3_b.astype(np.float32).reshape(D, 1),
    }

    O = np.asarray(inputs["O"], np.float32)  # [B, A, OBS]
    in_maps = []
    for c in range(NCORES):
        oc = O[c * BLOC : (c + 1) * BLOC]                  # [BLOC, A, OBS]
        # ot[h*64+f, st, a*512+s'] = O[st*1024 + h*512 + s', a, f]
        x = oc.reshape(NST, 2, ST // 2, A, OBS)
        ot = np.ascontiguousarray(x.transpose(1, 4, 0, 3, 2)).reshape(
            2 * OBS, NST, COLS // 2
        )
        in_maps.append({"ot": ot, **common})
    return in_maps


def build(repeats=1, nact=5):
    key = "nc%d_%d" % (repeats, nact)
    if key not in _compiled:
        _compiled[key] = _build_bass(repeats, nact)
    return _compiled[key]


def kernel(**inputs):
    nc = build(1)
    in_maps = _prep_inputs(inputs)
    res = run_bass_kernel_spmd(nc, in_maps, core_ids=list(range(NCORES)))
    # device layout [p, st*G*C]: sample (st*1024 + g*128 + p), class c sits
    # at [p, st*128 + g*16 + c] — undo on host
    outs = [
        res.results[i]["probs"]
        .reshape(D, NST, GROUPS, C)
        .transpose(1, 2, 0, 3)
        .reshape(BLOC, C)
        for i in range(NCORES)
    ]
    return np.ascontiguousarray(np.concatenate(outs, axis=0))

